# revision 1
# baseline (speedup 1.0000x reference)
"""Bionetwork sparse-matvec recurrence on 8 trn2 NeuronCores.

y_{t+1} = act(A y_t + b_in), 150 iterations, A fixed sparse (3.2M edges,
100k nodes).  Dest-sharded across 8 cores; all routing tables SBUF-resident.

Per iteration, per core (local_scatter = vectorized GPSIMD within-row scatter):
  1. seed-scatter per dest-chunk g: canonical y -> run-starts of expansion
  2. log-fill (DVE, masked shifted adds) completes source runs (len<=8)
  3. multiply by edge weights (fp16, in place)
  4. round-1 local_scatter: products -> staging tiles at col 128*t + dest_row
  5. PE transpose of each [128,128] staging tile (the cross-partition hop)
  6. round-2 local_scatter: transposed stream -> dest-slot layout
  7. segmented reduce (32-wide slots) -> fp32; fold pseudo-slot regions
  8. v = s + b_in; piecewise activation; write shard; AllGather; reload y

Everything is table-driven; tables are built host-side from the (fixed)
edge lists and shipped as per-core input tensors to one shared program.
"""
import numpy as np

N = 100000
E = 3200000
P = 128
NCORES = 8
QW = 800                    # canonical width: 128*800 = 102400
NC_PAD = P * QW
SHARD = NC_PAD // NCORES    # 12800 = 128*100
KMAX = SHARD // P           # 100
ITERS = 150
LEAK = 0.01
RUN_CAP = 16                # fill rounds 1,2,4,8 cover runs of 16
SEED_REGIONS = 1
MAX_DST = 2046
TILES_PER_CALL = 15
SD = SEED_REGIONS * QW


def _ceil(a, b):
    return -(-a // b)


def _prep(x, in_weights, rec_weights, biases, out_weights,
          in_indices, edge_rows, edge_cols, out_indices):
    deg = np.bincount(edge_rows, minlength=N)
    npseudo = np.maximum(1, _ceil(deg, 32))
    assert npseudo.max() <= 4, f"max in-degree {deg.max()} > 128 unsupported"

    # deal dests round-robin over 1024 (core,row) bins; sort by npseudo desc
    # (region contiguity) but shuffle within classes (chunk load balance)
    rng = np.random.default_rng(12345)
    order = np.lexsort((rng.permutation(N), -npseudo))
    i = np.arange(N)
    b = i % (NCORES * P)
    core_of, row_of, k_of = b % NCORES, b // NCORES, i // (NCORES * P)
    Kreal = int(k_of.max()) + 1
    assert Kreal <= KMAX
    perm = np.empty(N, np.int64)
    perm[order] = SHARD * core_of + KMAX * row_of + k_of

    nr_max = {r: _ceil(int((npseudo >= r).sum()), NCORES * P) for r in (2, 3, 4)}
    region_base = {1: 0}
    base = Kreal
    for r in (2, 3, 4):
        region_base[r] = base
        base += nr_max[r]
    KP = base
    FD = 32 * KP
    NCH = _ceil(FD, MAX_DST)
    CH = _ceil(_ceil(FD, NCH), 32) * 32
    NCH = _ceil(FD, CH)

    import jax.numpy as jnp
    node_in = np.asarray(
        jnp.zeros((N,), jnp.float32).at[jnp.asarray(in_indices)].set(
            jnp.asarray(in_weights, jnp.float32) * jnp.asarray(x[0], jnp.float32)))
    b_in_full = node_in + biases.astype(np.float32)

    dnew, snew = perm[edge_rows], perm[edge_cols]
    w_all = rec_weights.astype(np.float32)
    dcore = dnew // SHARD

    # ---------- pass 1: per-core edge geometry ----------
    geo = []
    for c in range(NCORES):
        em = np.where(dcore == c)[0]
        d_loc = dnew[em] - SHARD * c
        j, k = d_loc // KMAX, d_loc % KMAX
        s_new = snew[em]
        p0, q0 = s_new // QW, s_new % QW
        w = w_all[em]
        ne = em.size

        def ranks_of(key):
            so = np.argsort(key, kind="stable")
            ks = key[so]
            st = np.r_[0, np.flatnonzero(np.diff(ks)) + 1]
            sid = np.zeros(ne, np.int64)
            sid[st[1:]] = 1
            sid = np.cumsum(sid)
            r = np.arange(ne) - st[sid]
            out = np.empty(ne, np.int64)
            out[so] = r
            return out

        slot = ranks_of(d_loc)
        r_idx = slot // 32
        rbv = np.array([region_base[1], region_base[2], region_base[3], region_base[4]])
        f = 32 * (rbv[r_idx] + k) + slot % 32
        g = f // CH
        trank = ranks_of((g * P + p0) * P + j)
        # expansion position within (g,p0) ordered by q0, and rank within source
        so3 = np.lexsort((q0, p0, g))
        gp = (g * P + p0)[so3]
        st = np.r_[0, np.flatnonzero(np.diff(gp)) + 1]
        sid = np.zeros(ne, np.int64)
        sid[st[1:]] = 1
        sid = np.cumsum(sid)
        m_pos = np.empty(ne, np.int64)
        m_pos[so3] = np.arange(ne) - st[sid]
        gpq = ((g * P + p0) * QW + q0)[so3]
        st4 = np.r_[0, np.flatnonzero(np.diff(gpq)) + 1]
        sid4 = np.zeros(ne, np.int64)
        sid4[st4[1:]] = 1
        sid4 = np.cumsum(sid4)
        src_rank = np.empty(ne, np.int64)
        src_rank[so3] = np.arange(ne) - st4[sid4]
        assert int(src_rank.max()) < RUN_CAP * SEED_REGIONS
        geo.append(dict(j=j, p0=p0, q0=q0, w=w, f=f, g=g,
                        trank=trank, m_pos=m_pos, src_rank=src_rank, ne=ne))

    # uniform per-chunk sizes across cores
    M1 = np.zeros(NCH, np.int64)
    MTg = np.zeros(NCH, np.int64)
    for gg in geo:
        for g2 in range(NCH):
            sel = gg["g"] == g2
            if sel.any():
                M1[g2] = max(M1[g2], int(gg["m_pos"][sel].max()) + 1)
                MTg[g2] = max(MTg[g2], int(gg["trank"][sel].max()) + 1)
    M1 = (_ceil(M1, 2) * 2).astype(np.int64)
    EB = np.r_[0, np.cumsum(M1)]         # expansion bases
    MEXP = int(EB[-1])
    TBASE = np.r_[0, np.cumsum(MTg)]     # tile bases
    T = int(TBASE[-1])
    # round-1 call structure: (g, t0, t1) uniform
    r1_struct = []
    for g2 in range(NCH):
        for t0 in range(0, int(MTg[g2]), TILES_PER_CALL):
            r1_struct.append((g2, t0, min(t0 + TILES_PER_CALL, int(MTg[g2]))))
    NR1 = len(r1_struct)

    # ---------- pass 2: tables ----------
    cores = []
    for c in range(NCORES):
        gg = geo[c]
        j, p0, q0, w = gg["j"], gg["p0"], gg["q0"], gg["w"]
        f, g, trank, m_pos, src_rank = (gg["f"], gg["g"], gg["trank"],
                                        gg["m_pos"], gg["src_rank"])
        m_glob = EB[g] + m_pos
        dist = src_rank

        seedidx = np.full((NCH, P, SD), -1, np.int16)
        sm = dist == 0
        seedidx[g[sm], p0[sm], q0[sm]] = m_pos[sm].astype(np.int16)

        masks = np.zeros((4, P, MEXP), np.float16)
        for ki, kk in enumerate((1, 2, 4, 8)):
            mm = dist >= kk
            masks[ki, p0[mm], m_glob[mm]] = 1.0

        w_exp = np.zeros((P, MEXP), np.float16)
        w_exp[p0, m_glob] = w.astype(np.float16)

        idx1 = []
        for (g2, t0, t1) in r1_struct:
            sel = (g == g2) & (trank >= t0) & (trank < t1)
            idx = np.full((P, int(M1[g2])), -1, np.int16)
            idx[p0[sel], m_pos[sel]] = (128 * (trank[sel] - t0) + j[sel]).astype(np.int16)
            idx1.append(idx)

        idx2 = []
        for g2 in range(NCH):
            sel = g == g2
            idx = np.full((P, 128 * int(MTg[g2])), -1, np.int16)
            idx[j[sel], 128 * trank[sel] + p0[sel]] = (f[sel] - g2 * CH).astype(np.int16)
            idx2.append(idx)

        b_in_t = np.zeros((P, Kreal), np.float32)
        nid = np.where((perm >= SHARD * c) & (perm < SHARD * (c + 1)))[0]
        dl = perm[nid] - SHARD * c
        b_in_t[dl // KMAX, dl % KMAX] = b_in_full[nid]

        cores.append(dict(seedidx=seedidx, masks=masks, w_exp=w_exp,
                          idx1=idx1, idx2=idx2, b_in_t=b_in_t))

    meta = dict(Kreal=Kreal, KP=KP, FD=FD, NCH=NCH, CH=CH, M1=M1, EB=EB,
                MTg=MTg, TBASE=TBASE, T=T, MEXP=MEXP, NR1=NR1,
                r1_struct=r1_struct, nr_max=nr_max, region_base=region_base)
    return cores, perm, meta


def _act_np(v):
    y1 = np.maximum(v, np.float32(LEAK) * v)
    ysat = (1.0 - 0.25 / np.maximum(v, 0.5)).astype(v.dtype)
    return np.where(v > 0.5, ysat, y1)


def _sim(cores, perm, meta, n_iters, quant=True):
    dt = np.float16 if quant else np.float32
    Kreal, KP, FD, NCH, CH = (meta["Kreal"], meta["KP"], meta["FD"],
                              meta["NCH"], meta["CH"])
    M1, EB, MTg, TBASE, T, MEXP = (meta["M1"], meta["EB"], meta["MTg"],
                                   meta["TBASE"], meta["T"], meta["MEXP"])
    y = np.zeros(NC_PAD, np.float32)
    for it in range(n_iters):
        y2d = y.reshape(P, QW).astype(dt)
        seed_data = y2d
        y_next = np.zeros(NC_PAD, np.float32)
        for c, tb in enumerate(cores):
            exp_t = np.zeros((P, MEXP), dt)
            for g2 in range(NCH):
                sidx = tb["seedidx"][g2]
                pp, cc = np.where(sidx >= 0)
                exp_t[pp, EB[g2] + sidx[pp, cc]] = seed_data[pp, cc]
            for ki, kk in enumerate((1, 2, 4, 8)):
                sh = np.zeros_like(exp_t)
                sh[:, kk:] = exp_t[:, :-kk]
                exp_t = (exp_t + tb["masks"][ki].astype(dt) * sh).astype(dt)
            prod = (exp_t.astype(np.float32) * tb["w_exp"].astype(np.float32)).astype(dt)
            staging = np.zeros((P, 128 * T), dt)
            for ci, (g2, t0, t1) in enumerate(meta["r1_struct"]):
                idx = tb["idx1"][ci]
                data = prod[:, EB[g2]:EB[g2] + M1[g2]]
                pp, cc = np.where(idx >= 0)
                staging[pp, 128 * (TBASE[g2] + t0) + idx[pp, cc]] = data[pp, cc]
            t2 = np.zeros_like(staging)
            for t in range(T):
                t2[:, 128 * t:128 * (t + 1)] = staging[:, 128 * t:128 * (t + 1)].T
            slots = np.zeros((P, FD), dt)
            for g2 in range(NCH):
                idx = tb["idx2"][g2]
                data = t2[:, 128 * TBASE[g2]:128 * (TBASE[g2] + MTg[g2])]
                pp, cc = np.where(idx >= 0)
                slots[pp, g2 * CH + idx[pp, cc]] = data[pp, cc]
            sp = slots.reshape(P, KP, 32).astype(np.float32).sum(axis=2)
            s = sp[:, :Kreal].copy()
            for r in (2, 3, 4):
                nr = meta["nr_max"][r]
                if nr:
                    b0 = meta["region_base"][r]
                    s[:, :nr] += sp[:, b0:b0 + nr]
            v = s + tb["b_in_t"]
            y32 = _act_np(v)
            jj, kk2 = np.meshgrid(np.arange(P), np.arange(Kreal), indexing="ij")
            y_next[SHARD * c + KMAX * jj.ravel() + kk2.ravel()] = y32.ravel()
        y = y_next
    return y


# ============================ BASS KERNEL ============================

def _build(cores, meta, n_iters, no_cc=False):
    import concourse.bacc as bacc
    import concourse.mybir as mybir
    import concourse.tile as tile
    from concourse.masks import make_identity

    f16, f32, i16 = mybir.dt.float16, mybir.dt.float32, mybir.dt.int16
    AOP = mybir.AluOpType
    Kreal, KP, FD, NCH, CH = (meta["Kreal"], meta["KP"], meta["FD"],
                              meta["NCH"], meta["CH"])
    M1, EB, MTg, TBASE, T, MEXP, NR1 = (meta["M1"], meta["EB"], meta["MTg"],
                                        meta["TBASE"], meta["T"],
                                        meta["MEXP"], meta["NR1"])
    DSTW = [min(FD, (g + 1) * CH) - g * CH for g in range(NCH)]

    nc = bacc.Bacc("TRN2", target_bir_lowering=False)

    d_seed = [nc.dram_tensor(f"t_seed{g}", [P, SD], i16, kind="ExternalInput")
              for g in range(NCH)]
    d_mask = [nc.dram_tensor(f"t_mask{k}", [P, MEXP], f16, kind="ExternalInput")
              for k in range(4)]
    d_wexp = nc.dram_tensor("t_wexp", [P, MEXP], f16, kind="ExternalInput")
    d_idx1 = [nc.dram_tensor(f"t_idx1_{ci}", [P, int(M1[g2])], i16,
                             kind="ExternalInput")
              for ci, (g2, _, _) in enumerate(meta["r1_struct"])]
    d_idx2 = [nc.dram_tensor(f"t_idx2_{g}", [P, 128 * int(MTg[g])], i16,
                             kind="ExternalInput") for g in range(NCH)]
    d_bin = nc.dram_tensor("t_bin", [P, Kreal], f32, kind="ExternalInput")
    d_yout = nc.dram_tensor("y_out", [P, Kreal], f32, kind="ExternalOutput")
    d_ysh = nc.dram_tensor("y_shard", [1, SHARD], f16, kind="Internal")
    d_yfull = nc.dram_tensor("y_full", [1, NC_PAD], f16, kind="Internal",
                             addr_space="Shared")
    d_yin = nc.dram_tensor("y_in", [1, NC_PAD], f16, kind="ExternalInput")
    d_yall = nc.dram_tensor("y_all", [1, NC_PAD], f16, kind="ExternalOutput")

    with tile.TileContext(nc) as tc:
        with tc.tile_pool(name="tables", bufs=1) as tp, \
             tc.tile_pool(name="psum", bufs=8, space="PSUM") as pp:
            t_seed = [tp.tile([P, SD], i16, name=f"seed{g}") for g in range(NCH)]
            t_mask = [tp.tile([P, MEXP], f16, name=f"mask{k}") for k in range(4)]
            t_wexp = tp.tile([P, MEXP], f16, name="wexp")
            t_idx1 = [tp.tile([P, int(M1[g2])], i16, name=f"i1_{ci}")
                      for ci, (g2, _, _) in enumerate(meta["r1_struct"])]
            t_idx2 = [tp.tile([P, 128 * int(MTg[g])], i16, name=f"i2_{g}")
                      for g in range(NCH)]
            t_bin = tp.tile([P, Kreal], f32, name="bin")
            ident = tp.tile([P, P], f16, name="ident")
            y2d = tp.tile([P, QW], f16, name="y2d")
            expb = [tp.tile([P, int(M1[g])], f16, name=f"expb{g}")
                    for g in range(NCH)]
            tmpb = [tp.tile([P, int(M1[g])], f16, name=f"tmpb{g}")
                    for g in range(NCH)]
            stag = [tp.tile([P, 128 * int(MTg[g])], f16, name=f"stag{g}")
                    for g in range(NCH)]
            t2d = [tp.tile([P, 128 * int(MTg[g])], f16, name=f"t2d{g}")
                   for g in range(NCH)]
            slots = [tp.tile([P, DSTW[g] // 32, 32], f16, name=f"slots{g}")
                     for g in range(NCH)]
            sp = tp.tile([P, KP], f32, name="sp")
            vv = tp.tile([P, Kreal], f32, name="vv")
            y1b = tp.tile([P, Kreal], f32, name="y1b")
            rb = tp.tile([P, Kreal], f32, name="rb")
            mb = tp.tile([P, Kreal], f32, name="mb")
            y32 = tp.tile([P, Kreal], f32, name="y32")
            y16 = tp.tile([P, KMAX], f16, name="y16")

            for g in range(NCH):
                nc.sync.dma_start(t_seed[g][:], d_seed[g][:])
                nc.sync.dma_start(t_idx2[g][:], d_idx2[g][:])
            for k in range(4):
                nc.sync.dma_start(t_mask[k][:], d_mask[k][:])
            for ci in range(NR1):
                nc.sync.dma_start(t_idx1[ci][:], d_idx1[ci][:])
            nc.sync.dma_start(t_wexp[:], d_wexp[:])
            nc.sync.dma_start(t_bin[:], d_bin[:])
            make_identity(nc, ident[:])
            nc.sync.dma_start(y2d[:], d_yin[:].rearrange("o (p q) -> (o p) q", p=P))
            nc.vector.memset(y16[:], 0.0)

            r1_by_g = {}
            for ci, (g2, t0, t1) in enumerate(meta["r1_struct"]):
                r1_by_g.setdefault(g2, []).append((ci, t0, t1))

            def body(iv=None):
                for g in range(NCH):
                    w0, w1 = int(EB[g]), int(EB[g + 1])
                    mw = int(M1[g])
                    # expansion for chunk g
                    nc.gpsimd.local_scatter(
                        expb[g][:], y2d[:], t_seed[g][:],
                        channels=P, num_elems=mw, num_idxs=SD)
                    for ki, kk in enumerate((1, 2, 4, 8)):
                        nc.vector.memset(tmpb[g][:, 0:kk], 0.0)
                        nc.vector.tensor_tensor(
                            tmpb[g][:, kk:mw], expb[g][:, 0:mw - kk],
                            t_mask[ki][:, w0 + kk:w1], op=AOP.mult)
                        nc.vector.tensor_tensor(expb[g][:], expb[g][:],
                                                tmpb[g][:], op=AOP.add)
                    nc.vector.tensor_tensor(expb[g][:], expb[g][:],
                                            t_wexp[:, w0:w1], op=AOP.mult)
                    # round 1 into per-chunk staging
                    for ci, t0, t1 in r1_by_g[g]:
                        nt = t1 - t0
                        nc.gpsimd.local_scatter(
                            stag[g][:, 128 * t0:128 * t1], expb[g][:],
                            t_idx1[ci][:], channels=P, num_elems=128 * nt,
                            num_idxs=mw)
                    # transposes
                    Tg = int(MTg[g])
                    for tb0 in range(0, Tg, 8):
                        nb = min(8, Tg - tb0)
                        pt = pp.tile([P, 8 * P], f16, space="PSUM", tag="tr",
                                     name="tr")
                        for t in range(tb0, tb0 + nb):
                            nc.tensor.transpose(
                                pt[:, 128 * (t - tb0):128 * (t - tb0 + 1)],
                                stag[g][:, 128 * t:128 * (t + 1)], ident[:])
                        nc.scalar.copy(
                            t2d[g][:, 128 * tb0:128 * (tb0 + nb)],
                            pt[:, 0:128 * nb])
                    # round 2 into dest slots
                    nc.gpsimd.local_scatter(
                        slots[g][:].rearrange("p k s -> p (k s)"), t2d[g][:],
                        t_idx2[g][:], channels=P, num_elems=DSTW[g],
                        num_idxs=128 * Tg)
                    # segmented reduce for chunk g
                    c0 = g * CH // 32
                    nc.vector.tensor_reduce(
                        sp[:, c0:c0 + DSTW[g] // 32], slots[g][:],
                        axis=mybir.AxisListType.X, op=AOP.add)
                for r in (2, 3, 4):
                    nr = meta["nr_max"][r]
                    if nr:
                        b0 = meta["region_base"][r]
                        nc.vector.tensor_tensor(sp[:, 0:nr], sp[:, 0:nr],
                                                sp[:, b0:b0 + nr], op=AOP.add)
                nc.vector.tensor_tensor(vv[:], sp[:, 0:Kreal], t_bin[:], op=AOP.add)
                nc.vector.scalar_tensor_tensor(
                    y1b[:], vv[:], float(LEAK), vv[:], op0=AOP.mult, op1=AOP.max)
                nc.vector.tensor_scalar_max(rb[:], vv[:], 0.5)
                nc.vector.reciprocal(rb[:], rb[:])
                nc.vector.tensor_scalar(rb[:], rb[:], -0.25, 1.0,
                                        op0=AOP.mult, op1=AOP.add)
                nc.vector.tensor_scalar(mb[:], vv[:], 0.5, None, op0=AOP.is_gt)
                nc.vector.tensor_tensor(rb[:], rb[:], y1b[:], op=AOP.subtract)
                nc.vector.tensor_tensor(mb[:], mb[:], rb[:], op=AOP.mult)
                nc.vector.tensor_tensor(y32[:], y1b[:], mb[:], op=AOP.add)
                nc.vector.tensor_copy(y16[:, 0:Kreal], y32[:])
                nc.sync.dma_start(
                    d_ysh[:].rearrange("o (p k) -> (o p) k", p=P), y16[:])
                if not no_cc:
                    nc.gpsimd.collective_compute(
                        "AllGather", AOP.bypass,
                        replica_groups=[list(range(NCORES))],
                        ins=[d_ysh[:]], outs=[d_yfull[:]])
                nc.sync.dma_start(
                    y2d[:], d_yfull[:].rearrange("o (p q) -> (o p) q", p=P))

            for _ in range(n_iters):
                body()
            nc.sync.dma_start(d_yout[:], y32[:])
            nc.sync.dma_start(
                d_yall[:].rearrange("o (p q) -> (o p) q", p=P), y2d[:])

    nc.compile()
    return nc


def _in_maps(cores, meta):
    maps = []
    for tb in cores:
        m = {"t_wexp": tb["w_exp"], "t_bin": tb["b_in_t"]}
        for g in range(meta["NCH"]):
            m[f"t_seed{g}"] = tb["seedidx"][g]
            m[f"t_idx2_{g}"] = tb["idx2"][g]
        for k in range(4):
            m[f"t_mask{k}"] = np.ascontiguousarray(tb["masks"][k])
        for ci in range(meta["NR1"]):
            m[f"t_idx1_{ci}"] = tb["idx1"][ci]
        maps.append(m)
    return maps


def _gather_y(res, meta):
    Kreal = meta["Kreal"]
    y_full = np.zeros(NC_PAD, np.float32)
    jj, kk2 = np.meshgrid(np.arange(P), np.arange(Kreal), indexing="ij")
    for c in range(NCORES):
        y32 = res.results[c]["y_out"]
        y_full[SHARD * c + KMAX * jj.ravel() + kk2.ravel()] = y32.ravel()
    return y_full


SEG = 150  # whole run fits one NEFF


def kernel(**inputs):
    from concourse.bass_utils import run_bass_kernel_spmd
    inputs = {k: np.asarray(v) for k, v in inputs.items()}
    cores, perm, meta = _prep(**inputs)
    nseg = _ceil(ITERS, SEG)
    nc = _build(cores, meta, SEG)
    maps = _in_maps(cores, meta)
    y_state = np.zeros((1, NC_PAD), np.float16)
    res = None
    for s in range(nseg):
        for m in maps:
            m["y_in"] = y_state
        res = run_bass_kernel_spmd(nc, [dict(m) for m in maps],
                                   core_ids=list(range(NCORES)))
        y_state = res.results[0]["y_all"]
    y_old = _gather_y(res, meta)[perm]
    out = (inputs["out_weights"].astype(np.float32)
           * y_old[inputs["out_indices"]])[None, :]
    return out.astype(np.float32)


if __name__ == "__main__":
    import sys, time
    sys.path.insert(0, "/root/problem")
    import reference
    inputs = {k: np.asarray(v) for k, v in reference.setup_inputs().items()}
    t0 = time.time()
    cores, perm, meta = _prep(**inputs)
    print(f"prep {time.time()-t0:.1f}s Kreal={meta['Kreal']} KP={meta['KP']} "
          f"FD={meta['FD']} M1={meta['M1']} MTg={meta['MTg']} T={meta['T']} "
          f"MEXP={meta['MEXP']} NR1={meta['NR1']}")
    if "sim" in sys.argv:
        n_it = int(sys.argv[sys.argv.index("sim") + 1]) if len(sys.argv) > 2 else 8
        import jax.numpy as jnp
        ni = np.asarray(jnp.zeros((N,), jnp.float32).at[jnp.asarray(inputs["in_indices"])].set(
            jnp.asarray(inputs["in_weights"], jnp.float32) * jnp.asarray(inputs["x"][0], jnp.float32)))
        b_in = (ni + inputs["biases"]).astype(np.float64)
        rw = inputs["rec_weights"].astype(np.float64)
        er, ec = inputs["edge_rows"], inputs["edge_cols"]
        yref = np.zeros(N, np.float64)
        for _ in range(n_it):
            s = np.bincount(er, weights=rw * yref[ec], minlength=N)
            v = s + b_in
            yref = np.where(v > 0.5, 1.0 - 0.25 / np.maximum(v, 0.5),
                            np.maximum(v, LEAK * v))
        scale = np.abs(yref).max()
        t0 = time.time()
        ys = _sim(cores, perm, meta, n_it, quant=False)
        print(f"sim(noquant,{n_it}) {time.time()-t0:.1f}s  max rel err:",
              np.abs(ys[perm] - yref).max() / scale)
        t0 = time.time()
        ysq = _sim(cores, perm, meta, n_it, quant=True)
        print(f"sim(fp16,{n_it}) {time.time()-t0:.1f}s  max rel err:",
              np.abs(ysq[perm] - yref).max() / scale)



# revision 20
# speedup vs baseline: 1.1769x; 1.1769x over previous
"""Bionetwork sparse-matvec recurrence on 8 trn2 NeuronCores.

y_{t+1} = act(A y_t + b_in), 150 iterations, A fixed sparse (3.2M edges,
100k nodes).  Dest-sharded across 8 cores; all routing tables SBUF-resident.

Per iteration, per core (local_scatter = vectorized GPSIMD within-row scatter):
  1. seed-scatter per dest-chunk g: canonical y -> run-starts of expansion
  2. segmented forward-fill via one tensor_tensor_scan (state=mask*state+seed)
  3. multiply by edge weights (fp16, in place)
  4. round-1 local_scatter: products -> staging tiles at col 128*t + dest_row
  5. PE transpose of each [128,128] staging tile (the cross-partition hop)
  6. round-2 local_scatter: transposed stream -> dest-slot layout
  7. segmented reduce (32-wide slots, fp16); fold pseudo-slot regions
  8. v = s + b_in; piecewise activation (select-fused); AllGather; reload y

Everything is table-driven; tables are built host-side from the (fixed)
edge lists and shipped as per-core input tensors to one shared program.
"""
import numpy as np

N = 100000
E = 3200000
P = 128
NCORES = 8
QW = 800                    # canonical width: 128*800 = 102400
NC_PAD = P * QW
SHARD = NC_PAD // NCORES    # 12800 = 128*100
KMAX = SHARD // P           # 100
ITERS = 150
LEAK = 0.01
RUN_CAP = 16                # fill rounds 1,2,4,8 cover runs of 16
SEED_REGIONS = 1
MAX_DST = 2046
TILES_PER_CALL = 15
SD = SEED_REGIONS * QW


def _ceil(a, b):
    return -(-a // b)


def _prep(x, in_weights, rec_weights, biases, out_weights,
          in_indices, edge_rows, edge_cols, out_indices):
    deg = np.bincount(edge_rows, minlength=N)
    npseudo = np.maximum(1, _ceil(deg, 32))
    assert npseudo.max() <= 4, f"max in-degree {deg.max()} > 128 unsupported"

    # deal dests round-robin over 1024 (core,row) bins; sort by npseudo desc
    # (region contiguity) but shuffle within classes (chunk load balance)
    rng = np.random.default_rng(12345)
    order = np.lexsort((rng.permutation(N), -npseudo))
    i = np.arange(N)
    b = i % (NCORES * P)
    core_of, row_of, k_of = b % NCORES, b // NCORES, i // (NCORES * P)
    Kreal = int(k_of.max()) + 1
    assert Kreal <= KMAX
    perm = np.empty(N, np.int64)
    perm[order] = SHARD * core_of + KMAX * row_of + k_of

    nr_max = {r: _ceil(int((npseudo >= r).sum()), NCORES * P) for r in (2, 3, 4)}
    region_base = {1: 0}
    base = Kreal
    for r in (2, 3, 4):
        region_base[r] = base
        base += nr_max[r]
    KP = base
    FD = 32 * KP
    NCH = _ceil(FD, MAX_DST)
    CH = _ceil(_ceil(FD, NCH), 32) * 32
    NCH = _ceil(FD, CH)

    import jax.numpy as jnp
    node_in = np.asarray(
        jnp.zeros((N,), jnp.float32).at[jnp.asarray(in_indices)].set(
            jnp.asarray(in_weights, jnp.float32) * jnp.asarray(x[0], jnp.float32)))
    b_in_full = node_in + biases.astype(np.float32)

    dnew, snew = perm[edge_rows], perm[edge_cols]
    w_all = rec_weights.astype(np.float32)
    dcore = dnew // SHARD

    # ---------- pass 1: per-core edge geometry ----------
    geo = []
    for c in range(NCORES):
        em = np.where(dcore == c)[0]
        d_loc = dnew[em] - SHARD * c
        j, k = d_loc // KMAX, d_loc % KMAX
        s_new = snew[em]
        p0, q0 = s_new // QW, s_new % QW
        w = w_all[em]
        ne = em.size

        def ranks_of(key):
            so = np.argsort(key, kind="stable")
            ks = key[so]
            st = np.r_[0, np.flatnonzero(np.diff(ks)) + 1]
            sid = np.zeros(ne, np.int64)
            sid[st[1:]] = 1
            sid = np.cumsum(sid)
            r = np.arange(ne) - st[sid]
            out = np.empty(ne, np.int64)
            out[so] = r
            return out

        slot = ranks_of(d_loc)
        r_idx = slot // 32
        rbv = np.array([region_base[1], region_base[2], region_base[3], region_base[4]])
        f = 32 * (rbv[r_idx] + k) + slot % 32
        g = f // CH
        trank = ranks_of((g * P + p0) * P + j)
        # expansion position within (g,p0) ordered by q0, and rank within source
        so3 = np.lexsort((q0, p0, g))
        gp = (g * P + p0)[so3]
        st = np.r_[0, np.flatnonzero(np.diff(gp)) + 1]
        sid = np.zeros(ne, np.int64)
        sid[st[1:]] = 1
        sid = np.cumsum(sid)
        m_pos = np.empty(ne, np.int64)
        m_pos[so3] = np.arange(ne) - st[sid]
        gpq = ((g * P + p0) * QW + q0)[so3]
        st4 = np.r_[0, np.flatnonzero(np.diff(gpq)) + 1]
        sid4 = np.zeros(ne, np.int64)
        sid4[st4[1:]] = 1
        sid4 = np.cumsum(sid4)
        src_rank = np.empty(ne, np.int64)
        src_rank[so3] = np.arange(ne) - st4[sid4]
        geo.append(dict(j=j, p0=p0, q0=q0, w=w, f=f, g=g,
                        trank=trank, m_pos=m_pos, src_rank=src_rank, ne=ne))

    # uniform per-chunk sizes across cores
    M1 = np.zeros(NCH, np.int64)
    MTg = np.zeros(NCH, np.int64)
    for gg in geo:
        for g2 in range(NCH):
            sel = gg["g"] == g2
            if sel.any():
                M1[g2] = max(M1[g2], int(gg["m_pos"][sel].max()) + 1)
                MTg[g2] = max(MTg[g2], int(gg["trank"][sel].max()) + 1)
    M1 = (_ceil(M1, 2) * 2).astype(np.int64)
    EB = np.r_[0, np.cumsum(M1)]         # expansion bases
    MEXP = int(EB[-1])
    TBASE = np.r_[0, np.cumsum(MTg)]     # tile bases
    T = int(TBASE[-1])
    # round-1 call structure: (g, t0, t1), evenly-split windows <= 15 tiles
    r1_struct = []
    for g2 in range(NCH):
        tg = int(MTg[g2])
        ncall = _ceil(tg, TILES_PER_CALL)
        base, rem = divmod(tg, ncall)
        t0 = 0
        for ci in range(ncall):
            nt = base + (1 if ci < rem else 0)
            r1_struct.append((g2, t0, t0 + nt))
            t0 += nt
    NR1 = len(r1_struct)

    # ---------- pass 2: tables ----------
    cores = []
    for c in range(NCORES):
        gg = geo[c]
        j, p0, q0, w = gg["j"], gg["p0"], gg["q0"], gg["w"]
        f, g, trank, m_pos, src_rank = (gg["f"], gg["g"], gg["trank"],
                                        gg["m_pos"], gg["src_rank"])
        m_glob = EB[g] + m_pos
        dist = src_rank

        seedidx = np.full((NCH, P, SD), -1, np.int16)
        sm = dist == 0
        seedidx[g[sm], p0[sm], q0[sm]] = m_pos[sm].astype(np.int16)

        # scan fill mask: 1.0 inside a source run (copy state), 0.0 at starts
        runmask = np.zeros((P, MEXP), np.float16)
        mm = dist > 0
        runmask[p0[mm], m_glob[mm]] = 1.0

        w_exp = np.zeros((P, MEXP), np.float16)
        w_exp[p0, m_glob] = w.astype(np.float16)

        idx1 = []
        for (g2, t0, t1) in r1_struct:
            sel = (g == g2) & (trank >= t0) & (trank < t1)
            idx = np.full((P, int(M1[g2])), -1, np.int16)
            idx[p0[sel], m_pos[sel]] = (128 * (trank[sel] - t0) + j[sel]).astype(np.int16)
            idx1.append(idx)

        idx2 = []
        for g2 in range(NCH):
            sel = g == g2
            idx = np.full((P, 128 * int(MTg[g2])), -1, np.int16)
            idx[j[sel], 128 * trank[sel] + p0[sel]] = (f[sel] - g2 * CH).astype(np.int16)
            idx2.append(idx)

        b_in_t = np.zeros((P, Kreal), np.float32)
        nid = np.where((perm >= SHARD * c) & (perm < SHARD * (c + 1)))[0]
        dl = perm[nid] - SHARD * c
        b_in_t[dl // KMAX, dl % KMAX] = b_in_full[nid]

        cores.append(dict(seedidx=seedidx, runmask=runmask, w_exp=w_exp,
                          idx1=idx1, idx2=idx2, b_in_t=b_in_t))

    meta = dict(Kreal=Kreal, KP=KP, FD=FD, NCH=NCH, CH=CH, M1=M1, EB=EB,
                MTg=MTg, TBASE=TBASE, T=T, MEXP=MEXP, NR1=NR1,
                r1_struct=r1_struct, nr_max=nr_max, region_base=region_base)
    return cores, perm, meta


def _act_np(v):
    y1 = np.maximum(v, np.float32(LEAK) * v)
    ysat = (1.0 - 0.25 / np.maximum(v, 0.5)).astype(v.dtype)
    return np.where(v > 0.5, ysat, y1)


def _sim(cores, perm, meta, n_iters, quant=True):
    dt = np.float16 if quant else np.float32
    Kreal, KP, FD, NCH, CH = (meta["Kreal"], meta["KP"], meta["FD"],
                              meta["NCH"], meta["CH"])
    M1, EB, MTg, TBASE, T, MEXP = (meta["M1"], meta["EB"], meta["MTg"],
                                   meta["TBASE"], meta["T"], meta["MEXP"])
    y = np.zeros(NC_PAD, np.float32)
    for it in range(n_iters):
        y2d = y.reshape(P, QW).astype(dt)
        seed_data = y2d
        y_next = np.zeros(NC_PAD, np.float32)
        for c, tb in enumerate(cores):
            seeds = np.zeros((P, MEXP), dt)
            for g2 in range(NCH):
                sidx = tb["seedidx"][g2]
                pp, cc = np.where(sidx >= 0)
                seeds[pp, EB[g2] + sidx[pp, cc]] = seed_data[pp, cc]
            # segmented forward-fill scan: state = mask*state + seed (fp32
            # state, downcast per element) per chunk
            exp_t = np.zeros((P, MEXP), dt)
            rm = tb["runmask"].astype(np.float32)
            sd32 = seeds.astype(np.float32)
            for g2 in range(NCH):
                st = np.zeros(P, np.float32)
                for t in range(int(EB[g2]), int(EB[g2 + 1])):
                    st = rm[:, t] * st + sd32[:, t]
                    exp_t[:, t] = st.astype(dt)
            prod = (exp_t.astype(np.float32) * tb["w_exp"].astype(np.float32)).astype(dt)
            staging = np.zeros((P, 128 * T), dt)
            for ci, (g2, t0, t1) in enumerate(meta["r1_struct"]):
                idx = tb["idx1"][ci]
                data = prod[:, EB[g2]:EB[g2] + M1[g2]]
                pp, cc = np.where(idx >= 0)
                staging[pp, 128 * (TBASE[g2] + t0) + idx[pp, cc]] = data[pp, cc]
            t2 = np.zeros_like(staging)
            for t in range(T):
                t2[:, 128 * t:128 * (t + 1)] = staging[:, 128 * t:128 * (t + 1)].T
            slots = np.zeros((P, FD), dt)
            for g2 in range(NCH):
                idx = tb["idx2"][g2]
                data = t2[:, 128 * TBASE[g2]:128 * (TBASE[g2] + MTg[g2])]
                pp, cc = np.where(idx >= 0)
                slots[pp, g2 * CH + idx[pp, cc]] = data[pp, cc]
            sp = slots.reshape(P, KP, 32).astype(np.float32).sum(axis=2).astype(dt).astype(np.float32)
            s = sp[:, :Kreal].copy()
            for r in (2, 3, 4):
                nr = meta["nr_max"][r]
                if nr:
                    b0 = meta["region_base"][r]
                    s[:, :nr] += sp[:, b0:b0 + nr]
            v = s + tb["b_in_t"]
            y32 = _act_np(v)
            jj, kk2 = np.meshgrid(np.arange(P), np.arange(Kreal), indexing="ij")
            y_next[SHARD * c + KMAX * jj.ravel() + kk2.ravel()] = y32.ravel()
        y = y_next
    return y


# ============================ BASS KERNEL ============================

def _build(cores, meta, n_iters, no_cc=False):
    import concourse.bacc as bacc
    import concourse.mybir as mybir
    import concourse.tile as tile
    from concourse.masks import make_identity

    f16, f32, i16 = mybir.dt.float16, mybir.dt.float32, mybir.dt.int16
    AOP = mybir.AluOpType
    Kreal, KP, FD, NCH, CH = (meta["Kreal"], meta["KP"], meta["FD"],
                              meta["NCH"], meta["CH"])
    M1, EB, MTg, TBASE, T, MEXP, NR1 = (meta["M1"], meta["EB"], meta["MTg"],
                                        meta["TBASE"], meta["T"],
                                        meta["MEXP"], meta["NR1"])
    DSTW = [min(FD, (g + 1) * CH) - g * CH for g in range(NCH)]

    nc = bacc.Bacc("TRN2", target_bir_lowering=False)

    d_seed = [nc.dram_tensor(f"t_seed{g}", [P, SD], i16, kind="ExternalInput")
              for g in range(NCH)]
    d_rmask = nc.dram_tensor("t_rmask", [P, MEXP], f16, kind="ExternalInput")
    d_wexp = nc.dram_tensor("t_wexp", [P, MEXP], f16, kind="ExternalInput")
    d_idx1 = [nc.dram_tensor(f"t_idx1_{ci}", [P, int(M1[g2])], i16,
                             kind="ExternalInput")
              for ci, (g2, _, _) in enumerate(meta["r1_struct"])]
    d_idx2 = [nc.dram_tensor(f"t_idx2_{g}", [P, 128 * int(MTg[g])], i16,
                             kind="ExternalInput") for g in range(NCH)]
    d_bin = nc.dram_tensor("t_bin", [P, Kreal], f32, kind="ExternalInput")
    d_yout = nc.dram_tensor("y_out", [P, Kreal], f16, kind="ExternalOutput")
    d_ysh = nc.dram_tensor("y_shard", [1, SHARD], f16, kind="Internal")
    d_yfull = nc.dram_tensor("y_full", [1, NC_PAD], f16, kind="Internal",
                             addr_space="Shared")
    d_yin = nc.dram_tensor("y_in", [1, NC_PAD], f16, kind="ExternalInput")
    d_yall = nc.dram_tensor("y_all", [1, NC_PAD], f16, kind="ExternalOutput")

    with tile.TileContext(nc) as tc:
        with tc.tile_pool(name="tables", bufs=1) as tp, \
             tc.tile_pool(name="psum", bufs=8, space="PSUM") as pp:
            t_seed = [tp.tile([P, SD], i16, name=f"seed{g}") for g in range(NCH)]
            t_rmask = tp.tile([P, MEXP], f16, name="rmask")
            t_wexp = tp.tile([P, MEXP], f16, name="wexp")
            t_idx1 = [tp.tile([P, int(M1[g2])], i16, name=f"i1_{ci}")
                      for ci, (g2, _, _) in enumerate(meta["r1_struct"])]
            t_idx2 = [tp.tile([P, 128 * int(MTg[g])], i16, name=f"i2_{g}")
                      for g in range(NCH)]
            t_bin = tp.tile([P, Kreal], f32, name="bin")
            ident = tp.tile([P, P], f16, name="ident")
            y2d = tp.tile([P, QW], f16, name="y2d")
            expb = [tp.tile([P, int(M1[g])], f16, name=f"expb{g}")
                    for g in range(NCH)]
            seedb = [tp.tile([P, int(M1[g])], f16, name=f"seedb{g}")
                     for g in range(NCH)]
            stag = [tp.tile([P, 128 * int(MTg[g])], f16, name=f"stag{g}")
                    for g in range(NCH)]
            t2d = [tp.tile([P, 128 * int(MTg[g])], f16, name=f"t2d{g}")
                   for g in range(NCH)]
            slots = [tp.tile([P, DSTW[g] // 32, 32], f16, name=f"slots{g}")
                     for g in range(NCH)]
            sp = tp.tile([P, KP], f16, name="sp")
            vv = tp.tile([P, Kreal], f32, name="vv")
            y1b = tp.tile([P, Kreal], f32, name="y1b")
            rb = tp.tile([P, Kreal], f32, name="rb")
            mb = tp.tile([P, Kreal], mybir.dt.uint8, name="mb")
            y16 = tp.tile([P, KMAX], f16, name="y16")

            for g in range(NCH):
                nc.sync.dma_start(t_seed[g][:], d_seed[g][:])
                nc.sync.dma_start(t_idx2[g][:], d_idx2[g][:])
            nc.sync.dma_start(t_rmask[:], d_rmask[:])
            for ci in range(NR1):
                nc.sync.dma_start(t_idx1[ci][:], d_idx1[ci][:])
            nc.sync.dma_start(t_wexp[:], d_wexp[:])
            nc.sync.dma_start(t_bin[:], d_bin[:])
            make_identity(nc, ident[:])
            nc.sync.dma_start(y2d[:], d_yin[:].rearrange("o (p q) -> (o p) q", p=P))
            nc.vector.memset(y16[:], 0.0)

            r1_by_g = {}
            for ci, (g2, t0, t1) in enumerate(meta["r1_struct"]):
                r1_by_g.setdefault(g2, []).append((ci, t0, t1))

            def body(iv=None):
                for g in range(NCH):
                    w0, w1 = int(EB[g]), int(EB[g + 1])
                    mw = int(M1[g])
                    # seed run-starts for chunk g, then segmented forward-fill
                    nc.gpsimd.local_scatter(
                        seedb[g][:], y2d[:], t_seed[g][:],
                        channels=P, num_elems=mw, num_idxs=SD)
                    nc.vector.tensor_tensor_scan(
                        expb[g][:], t_rmask[:, w0:w1], seedb[g][:], 0.0,
                        op0=AOP.mult, op1=AOP.add)
                    nc.vector.tensor_tensor(expb[g][:], expb[g][:],
                                            t_wexp[:, w0:w1], op=AOP.mult)
                    # round 1 into per-chunk staging
                    for ci, t0, t1 in r1_by_g[g]:
                        nt = t1 - t0
                        nc.gpsimd.local_scatter(
                            stag[g][:, 128 * t0:128 * t1], expb[g][:],
                            t_idx1[ci][:], channels=P, num_elems=128 * nt,
                            num_idxs=mw)
                    # transposes
                    Tg = int(MTg[g])
                    for tb0 in range(0, Tg, 8):
                        nb = min(8, Tg - tb0)
                        pt = pp.tile([P, 8 * P], f16, space="PSUM", tag="tr",
                                     name="tr")
                        for t in range(tb0, tb0 + nb):
                            nc.tensor.transpose(
                                pt[:, 128 * (t - tb0):128 * (t - tb0 + 1)],
                                stag[g][:, 128 * t:128 * (t + 1)], ident[:])
                        nc.scalar.copy(
                            t2d[g][:, 128 * tb0:128 * (tb0 + nb)],
                            pt[:, 0:128 * nb])
                    # round 2 into dest slots
                    nc.gpsimd.local_scatter(
                        slots[g][:].rearrange("p k s -> p (k s)"), t2d[g][:],
                        t_idx2[g][:], channels=P, num_elems=DSTW[g],
                        num_idxs=128 * Tg)
                    # segmented reduce for chunk g (fp16 out: values |w*y|<0.2,
                    # 32-wide sums stay O(1); validated vs fp64 reference)
                    c0 = g * CH // 32
                    with nc.allow_low_precision(reason="32-wide fp16 slot sums"):
                        nc.vector.tensor_reduce(
                            sp[:, c0:c0 + DSTW[g] // 32], slots[g][:],
                            axis=mybir.AxisListType.X, op=AOP.add)
                for r in (2, 3, 4):
                    nr = meta["nr_max"][r]
                    if nr:
                        b0 = meta["region_base"][r]
                        nc.vector.tensor_tensor(sp[:, 0:nr], sp[:, 0:nr],
                                                sp[:, b0:b0 + nr], op=AOP.add)
                nc.vector.tensor_tensor(vv[:], sp[:, 0:Kreal], t_bin[:], op=AOP.add)
                nc.vector.scalar_tensor_tensor(
                    y1b[:], vv[:], float(LEAK), vv[:], op0=AOP.mult, op1=AOP.max)
                nc.vector.tensor_scalar_max(rb[:], vv[:], 0.5)
                nc.vector.reciprocal(rb[:], rb[:])
                nc.vector.tensor_scalar(rb[:], rb[:], -0.25, 1.0,
                                        op0=AOP.mult, op1=AOP.add)
                nc.vector.tensor_scalar(mb[:], vv[:], 0.5, None, op0=AOP.is_gt)
                nc.vector.select(y16[:, 0:Kreal], mb[:], rb[:], y1b[:])
                nc.sync.dma_start(
                    d_ysh[:].rearrange("o (p k) -> (o p) k", p=P), y16[:])
                if not no_cc:
                    nc.gpsimd.collective_compute(
                        "AllGather", AOP.bypass,
                        replica_groups=[list(range(NCORES))],
                        ins=[d_ysh[:]], outs=[d_yfull[:]])
                nc.sync.dma_start(
                    y2d[:], d_yfull[:].rearrange("o (p q) -> (o p) q", p=P))

            for _ in range(n_iters):
                body()
            nc.sync.dma_start(d_yout[:], y16[:, 0:Kreal])
            nc.sync.dma_start(
                d_yall[:].rearrange("o (p q) -> (o p) q", p=P), y2d[:])

    nc.compile()
    return nc


def _in_maps(cores, meta):
    maps = []
    for tb in cores:
        m = {"t_wexp": tb["w_exp"], "t_bin": tb["b_in_t"],
             "t_rmask": tb["runmask"]}
        for g in range(meta["NCH"]):
            m[f"t_seed{g}"] = tb["seedidx"][g]
            m[f"t_idx2_{g}"] = tb["idx2"][g]
        for ci in range(meta["NR1"]):
            m[f"t_idx1_{ci}"] = tb["idx1"][ci]
        maps.append(m)
    return maps


def _gather_y(res, meta):
    Kreal = meta["Kreal"]
    y_full = np.zeros(NC_PAD, np.float32)
    jj, kk2 = np.meshgrid(np.arange(P), np.arange(Kreal), indexing="ij")
    for c in range(NCORES):
        y32 = res.results[c]["y_out"]
        y_full[SHARD * c + KMAX * jj.ravel() + kk2.ravel()] = y32.ravel()
    return y_full


SEG = 150  # whole run fits one NEFF


def kernel(**inputs):
    from concourse.bass_utils import run_bass_kernel_spmd
    inputs = {k: np.asarray(v) for k, v in inputs.items()}
    cores, perm, meta = _prep(**inputs)
    nseg = _ceil(ITERS, SEG)
    nc = _build(cores, meta, SEG)
    maps = _in_maps(cores, meta)
    y_state = np.zeros((1, NC_PAD), np.float16)
    res = None
    for s in range(nseg):
        for m in maps:
            m["y_in"] = y_state
        res = run_bass_kernel_spmd(nc, [dict(m) for m in maps],
                                   core_ids=list(range(NCORES)))
        y_state = res.results[0]["y_all"]
    y_old = _gather_y(res, meta)[perm]
    out = (inputs["out_weights"].astype(np.float32)
           * y_old[inputs["out_indices"]])[None, :]
    return out.astype(np.float32)


if __name__ == "__main__":
    import sys, time
    sys.path.insert(0, "/root/problem")
    import reference
    inputs = {k: np.asarray(v) for k, v in reference.setup_inputs().items()}
    t0 = time.time()
    cores, perm, meta = _prep(**inputs)
    print(f"prep {time.time()-t0:.1f}s Kreal={meta['Kreal']} KP={meta['KP']} "
          f"FD={meta['FD']} M1={meta['M1']} MTg={meta['MTg']} T={meta['T']} "
          f"MEXP={meta['MEXP']} NR1={meta['NR1']}")
    if "sim" in sys.argv:
        n_it = int(sys.argv[sys.argv.index("sim") + 1]) if len(sys.argv) > 2 else 8
        import jax.numpy as jnp
        ni = np.asarray(jnp.zeros((N,), jnp.float32).at[jnp.asarray(inputs["in_indices"])].set(
            jnp.asarray(inputs["in_weights"], jnp.float32) * jnp.asarray(inputs["x"][0], jnp.float32)))
        b_in = (ni + inputs["biases"]).astype(np.float64)
        rw = inputs["rec_weights"].astype(np.float64)
        er, ec = inputs["edge_rows"], inputs["edge_cols"]
        yref = np.zeros(N, np.float64)
        for _ in range(n_it):
            s = np.bincount(er, weights=rw * yref[ec], minlength=N)
            v = s + b_in
            yref = np.where(v > 0.5, 1.0 - 0.25 / np.maximum(v, 0.5),
                            np.maximum(v, LEAK * v))
        scale = np.abs(yref).max()
        t0 = time.time()
        ys = _sim(cores, perm, meta, n_it, quant=False)
        print(f"sim(noquant,{n_it}) {time.time()-t0:.1f}s  max rel err:",
              np.abs(ys[perm] - yref).max() / scale)
        t0 = time.time()
        ysq = _sim(cores, perm, meta, n_it, quant=True)
        print(f"sim(fp16,{n_it}) {time.time()-t0:.1f}s  max rel err:",
              np.abs(ysq[perm] - yref).max() / scale)



# revision 26
# speedup vs baseline: 1.2376x; 1.0516x over previous
"""Bionetwork sparse-matvec recurrence on 8 trn2 NeuronCores.

y_{t+1} = act(A y_t + b_in), 150 iterations, A fixed sparse (3.2M edges,
100k nodes).  Dest-sharded across 8 cores; all routing tables SBUF-resident.

Per iteration, per core (local_scatter = vectorized GPSIMD within-row scatter):
  1. seed-scatter per dest-chunk g: canonical y -> run-starts of expansion
  2. segmented forward-fill via one tensor_tensor_scan (state=mask*state+seed)
  3. multiply by edge weights (fp16, in place)
  4. round-1 local_scatter: products -> staging tiles at col 128*t + dest_row
  5. PE transpose of each [128,128] staging tile (the cross-partition hop)
  6. round-2 local_scatter: transposed stream -> dest-slot layout
  7. segmented reduce (32-wide slots, fp16); fold pseudo-slot regions
  8. v = s + b_in; piecewise activation (select-fused); AllGather; reload y

Everything is table-driven; tables are built host-side from the (fixed)
edge lists and shipped as per-core input tensors to one shared program.
"""
import numpy as np

N = 100000
E = 3200000
P = 128
NCORES = 8
QW = 800                    # canonical width: 128*800 = 102400
NC_PAD = P * QW
SHARD = NC_PAD // NCORES    # 12800 = 128*100
KMAX = SHARD // P           # 100
ITERS = 150
LEAK = 0.01
RUN_CAP = 16                # fill rounds 1,2,4,8 cover runs of 16
SEED_REGIONS = 1
MAX_DST = 2046
TILES_PER_CALL = 15
SD = SEED_REGIONS * QW


def _ceil(a, b):
    return -(-a // b)


def _prep(x, in_weights, rec_weights, biases, out_weights,
          in_indices, edge_rows, edge_cols, out_indices):
    deg = np.bincount(edge_rows, minlength=N)
    npseudo = np.maximum(1, _ceil(deg, 32))
    assert npseudo.max() <= 4, f"max in-degree {deg.max()} > 128 unsupported"

    # deal dests round-robin over 1024 (core,row) bins; sort by npseudo desc
    # (region contiguity) but shuffle within classes (chunk load balance)
    rng = np.random.default_rng(12345)
    order = np.lexsort((rng.permutation(N), -npseudo))
    i = np.arange(N)
    b = i % (NCORES * P)
    core_of, row_of, k_of = b % NCORES, b // NCORES, i // (NCORES * P)
    Kreal = int(k_of.max()) + 1
    assert Kreal <= KMAX
    perm = np.empty(N, np.int64)
    perm[order] = SHARD * core_of + KMAX * row_of + k_of

    nr_max = {r: _ceil(int((npseudo >= r).sum()), NCORES * P) for r in (2, 3, 4)}
    region_base = {1: 0}
    base = Kreal
    for r in (2, 3, 4):
        region_base[r] = base
        base += nr_max[r]
    KP = base
    FD = 32 * KP
    NCH = _ceil(FD, MAX_DST)
    CH = _ceil(_ceil(FD, NCH), 32) * 32
    NCH = _ceil(FD, CH)

    import jax.numpy as jnp
    node_in = np.asarray(
        jnp.zeros((N,), jnp.float32).at[jnp.asarray(in_indices)].set(
            jnp.asarray(in_weights, jnp.float32) * jnp.asarray(x[0], jnp.float32)))
    b_in_full = node_in + biases.astype(np.float32)

    dnew, snew = perm[edge_rows], perm[edge_cols]
    w_all = rec_weights.astype(np.float32)
    dcore = dnew // SHARD

    # ---------- pass 1: per-core edge geometry ----------
    geo = []
    for c in range(NCORES):
        em = np.where(dcore == c)[0]
        d_loc = dnew[em] - SHARD * c
        j, k = d_loc // KMAX, d_loc % KMAX
        s_new = snew[em]
        p0, q0 = s_new // QW, s_new % QW
        w = w_all[em]
        ne = em.size

        def ranks_of(key):
            so = np.argsort(key, kind="stable")
            ks = key[so]
            st = np.r_[0, np.flatnonzero(np.diff(ks)) + 1]
            sid = np.zeros(ne, np.int64)
            sid[st[1:]] = 1
            sid = np.cumsum(sid)
            r = np.arange(ne) - st[sid]
            out = np.empty(ne, np.int64)
            out[so] = r
            return out

        slot = ranks_of(d_loc)
        r_idx = slot // 32
        rbv = np.array([region_base[1], region_base[2], region_base[3], region_base[4]])
        f = 32 * (rbv[r_idx] + k) + slot % 32
        g = f // CH
        trank = ranks_of((g * P + p0) * P + j)
        # expansion position within (g,p0) ordered by q0, and rank within source
        so3 = np.lexsort((q0, p0, g))
        gp = (g * P + p0)[so3]
        st = np.r_[0, np.flatnonzero(np.diff(gp)) + 1]
        sid = np.zeros(ne, np.int64)
        sid[st[1:]] = 1
        sid = np.cumsum(sid)
        m_pos = np.empty(ne, np.int64)
        m_pos[so3] = np.arange(ne) - st[sid]
        gpq = ((g * P + p0) * QW + q0)[so3]
        st4 = np.r_[0, np.flatnonzero(np.diff(gpq)) + 1]
        sid4 = np.zeros(ne, np.int64)
        sid4[st4[1:]] = 1
        sid4 = np.cumsum(sid4)
        src_rank = np.empty(ne, np.int64)
        src_rank[so3] = np.arange(ne) - st4[sid4]
        geo.append(dict(j=j, p0=p0, q0=q0, w=w, f=f, g=g,
                        trank=trank, m_pos=m_pos, src_rank=src_rank, ne=ne))

    # uniform per-chunk sizes across cores
    M1 = np.zeros(NCH, np.int64)
    MTg = np.zeros(NCH, np.int64)
    for gg in geo:
        for g2 in range(NCH):
            sel = gg["g"] == g2
            if sel.any():
                M1[g2] = max(M1[g2], int(gg["m_pos"][sel].max()) + 1)
                MTg[g2] = max(MTg[g2], int(gg["trank"][sel].max()) + 1)
    M1 = (_ceil(M1, 2) * 2).astype(np.int64)
    EB = np.r_[0, np.cumsum(M1)]         # expansion bases
    MEXP = int(EB[-1])
    TBASE = np.r_[0, np.cumsum(MTg)]     # tile bases
    T = int(TBASE[-1])
    # round-1 call structure: (g, t0, t1), evenly-split windows <= 15 tiles
    r1_struct = []
    for g2 in range(NCH):
        tg = int(MTg[g2])
        ncall = _ceil(tg, TILES_PER_CALL)
        base, rem = divmod(tg, ncall)
        t0 = 0
        for ci in range(ncall):
            nt = base + (1 if ci < rem else 0)
            r1_struct.append((g2, t0, t0 + nt))
            t0 += nt
    NR1 = len(r1_struct)

    # ---------- pass 2: tables ----------
    cores = []
    for c in range(NCORES):
        gg = geo[c]
        j, p0, q0, w = gg["j"], gg["p0"], gg["q0"], gg["w"]
        f, g, trank, m_pos, src_rank = (gg["f"], gg["g"], gg["trank"],
                                        gg["m_pos"], gg["src_rank"])
        m_glob = EB[g] + m_pos
        dist = src_rank

        seedidx = np.full((NCH, P, SD), -1, np.int16)
        sm = dist == 0
        seedidx[g[sm], p0[sm], q0[sm]] = m_pos[sm].astype(np.int16)

        # scan fill mask: 1.0 inside a source run (copy state), 0.0 at starts
        runmask = np.zeros((P, MEXP), np.float16)
        mm = dist > 0
        runmask[p0[mm], m_glob[mm]] = 1.0

        w_exp = np.zeros((P, MEXP), np.float16)
        w_exp[p0, m_glob] = w.astype(np.float16)

        idx1 = []
        for (g2, t0, t1) in r1_struct:
            sel = (g == g2) & (trank >= t0) & (trank < t1)
            idx = np.full((P, int(M1[g2])), -1, np.int16)
            idx[p0[sel], m_pos[sel]] = (128 * (trank[sel] - t0) + j[sel]).astype(np.int16)
            idx1.append(idx)

        idx2 = []
        for g2 in range(NCH):
            sel = g == g2
            idx = np.full((P, 128 * int(MTg[g2])), -1, np.int16)
            idx[j[sel], 128 * trank[sel] + p0[sel]] = (f[sel] - g2 * CH).astype(np.int16)
            idx2.append(idx)

        b_in_t = np.zeros((P, Kreal), np.float32)
        nid = np.where((perm >= SHARD * c) & (perm < SHARD * (c + 1)))[0]
        dl = perm[nid] - SHARD * c
        b_in_t[dl // KMAX, dl % KMAX] = b_in_full[nid]

        cores.append(dict(seedidx=seedidx, runmask=runmask, w_exp=w_exp,
                          idx1=idx1, idx2=idx2, b_in_t=b_in_t))

    meta = dict(Kreal=Kreal, KP=KP, FD=FD, NCH=NCH, CH=CH, M1=M1, EB=EB,
                MTg=MTg, TBASE=TBASE, T=T, MEXP=MEXP, NR1=NR1,
                r1_struct=r1_struct, nr_max=nr_max, region_base=region_base)
    return cores, perm, meta


def _act_np(v):
    y1 = np.maximum(v, np.float32(LEAK) * v)
    ysat = (1.0 - 0.25 / np.maximum(v, 0.5)).astype(v.dtype)
    return np.where(v > 0.5, ysat, y1)


def _sim(cores, perm, meta, n_iters, quant=True):
    dt = np.float16 if quant else np.float32
    Kreal, KP, FD, NCH, CH = (meta["Kreal"], meta["KP"], meta["FD"],
                              meta["NCH"], meta["CH"])
    M1, EB, MTg, TBASE, T, MEXP = (meta["M1"], meta["EB"], meta["MTg"],
                                   meta["TBASE"], meta["T"], meta["MEXP"])
    y = np.zeros(NC_PAD, np.float32)
    for it in range(n_iters):
        y2d = y.reshape(P, QW).astype(dt)
        seed_data = y2d
        y_next = np.zeros(NC_PAD, np.float32)
        for c, tb in enumerate(cores):
            seeds = np.zeros((P, MEXP), dt)
            for g2 in range(NCH):
                sidx = tb["seedidx"][g2]
                pp, cc = np.where(sidx >= 0)
                seeds[pp, EB[g2] + sidx[pp, cc]] = seed_data[pp, cc]
            # segmented forward-fill scan: state = mask*state + seed (fp32
            # state, downcast per element) per chunk
            exp_t = np.zeros((P, MEXP), dt)
            rm = tb["runmask"].astype(np.float32)
            sd32 = seeds.astype(np.float32)
            for g2 in range(NCH):
                st = np.zeros(P, np.float32)
                for t in range(int(EB[g2]), int(EB[g2 + 1])):
                    st = rm[:, t] * st + sd32[:, t]
                    exp_t[:, t] = st.astype(dt)
            prod = (exp_t.astype(np.float32) * tb["w_exp"].astype(np.float32)).astype(dt)
            staging = np.zeros((P, 128 * T), dt)
            for ci, (g2, t0, t1) in enumerate(meta["r1_struct"]):
                idx = tb["idx1"][ci]
                data = prod[:, EB[g2]:EB[g2] + M1[g2]]
                pp, cc = np.where(idx >= 0)
                staging[pp, 128 * (TBASE[g2] + t0) + idx[pp, cc]] = data[pp, cc]
            t2 = np.zeros_like(staging)
            for t in range(T):
                t2[:, 128 * t:128 * (t + 1)] = staging[:, 128 * t:128 * (t + 1)].T
            slots = np.zeros((P, FD), dt)
            for g2 in range(NCH):
                idx = tb["idx2"][g2]
                data = t2[:, 128 * TBASE[g2]:128 * (TBASE[g2] + MTg[g2])]
                pp, cc = np.where(idx >= 0)
                slots[pp, g2 * CH + idx[pp, cc]] = data[pp, cc]
            sp = slots.reshape(P, KP, 32).astype(np.float32).sum(axis=2).astype(dt).astype(np.float32)
            s = sp[:, :Kreal].copy()
            for r in (2, 3, 4):
                nr = meta["nr_max"][r]
                if nr:
                    b0 = meta["region_base"][r]
                    s[:, :nr] += sp[:, b0:b0 + nr]
            v = s + tb["b_in_t"]
            y32 = _act_np(v)
            jj, kk2 = np.meshgrid(np.arange(P), np.arange(Kreal), indexing="ij")
            y_next[SHARD * c + KMAX * jj.ravel() + kk2.ravel()] = y32.ravel()
        y = y_next
    return y


# ============================ BASS KERNEL ============================

def _build(cores, meta, n_iters, no_cc=False, skip_last_exchange=False):
    import concourse.bacc as bacc
    import concourse.mybir as mybir
    import concourse.tile as tile
    from concourse.masks import make_identity

    f16, f32, i16 = mybir.dt.float16, mybir.dt.float32, mybir.dt.int16
    AOP = mybir.AluOpType
    Kreal, KP, FD, NCH, CH = (meta["Kreal"], meta["KP"], meta["FD"],
                              meta["NCH"], meta["CH"])
    M1, EB, MTg, TBASE, T, MEXP, NR1 = (meta["M1"], meta["EB"], meta["MTg"],
                                        meta["TBASE"], meta["T"],
                                        meta["MEXP"], meta["NR1"])
    DSTW = [min(FD, (g + 1) * CH) - g * CH for g in range(NCH)]

    nc = bacc.Bacc("TRN2", target_bir_lowering=False)

    d_seed = [nc.dram_tensor(f"t_seed{g}", [P, SD], i16, kind="ExternalInput")
              for g in range(NCH)]
    d_rmask = nc.dram_tensor("t_rmask", [P, MEXP], f16, kind="ExternalInput")
    d_wexp = nc.dram_tensor("t_wexp", [P, MEXP], f16, kind="ExternalInput")
    d_idx1 = [nc.dram_tensor(f"t_idx1_{ci}", [P, int(M1[g2])], i16,
                             kind="ExternalInput")
              for ci, (g2, _, _) in enumerate(meta["r1_struct"])]
    d_idx2 = [nc.dram_tensor(f"t_idx2_{g}", [P, 128 * int(MTg[g])], i16,
                             kind="ExternalInput") for g in range(NCH)]
    d_bin = nc.dram_tensor("t_bin", [P, Kreal], f32, kind="ExternalInput")
    d_yout = nc.dram_tensor("y_out", [P, Kreal], f16, kind="ExternalOutput")
    d_ysh = nc.dram_tensor("y_shard", [1, SHARD], f16, kind="Internal")
    d_yfull = nc.dram_tensor("y_full", [1, NC_PAD], f16, kind="Internal",
                             addr_space="Shared")
    d_yin = nc.dram_tensor("y_in", [1, NC_PAD], f16, kind="ExternalInput")
    d_yall = nc.dram_tensor("y_all", [1, NC_PAD], f16, kind="ExternalOutput")

    with tile.TileContext(nc) as tc:
        with tc.tile_pool(name="tables", bufs=1) as tp, \
             tc.tile_pool(name="psum", bufs=8, space="PSUM") as pp:
            t_seed = [tp.tile([P, SD], i16, name=f"seed{g}") for g in range(NCH)]
            t_rmask = tp.tile([P, MEXP], f16, name="rmask")
            t_wexp = tp.tile([P, MEXP], f16, name="wexp")
            t_idx1 = [tp.tile([P, int(M1[g2])], i16, name=f"i1_{ci}")
                      for ci, (g2, _, _) in enumerate(meta["r1_struct"])]
            t_idx2 = [tp.tile([P, 128 * int(MTg[g])], i16, name=f"i2_{g}")
                      for g in range(NCH)]
            t_bin = tp.tile([P, Kreal], f32, name="bin")
            ident = tp.tile([P, P], f16, name="ident")
            y2d = tp.tile([P, QW], f16, name="y2d")
            expb = [tp.tile([P, int(M1[g])], f16, name=f"expb{g}")
                    for g in range(NCH)]
            seedb = [tp.tile([P, int(M1[g])], f16, name=f"seedb{g}")
                     for g in range(NCH)]
            stag = [tp.tile([P, 128 * int(MTg[g])], f16, name=f"stag{g}")
                    for g in range(NCH)]
            t2d = [tp.tile([P, 128 * int(MTg[g])], f16, name=f"t2d{g}")
                   for g in range(NCH)]
            slots = [tp.tile([P, DSTW[g] // 32, 32], f16, name=f"slots{g}")
                     for g in range(NCH)]
            sp = tp.tile([P, KP], f16, name="sp")
            vv = tp.tile([P, Kreal], f32, name="vv")
            y1b = tp.tile([P, Kreal], f32, name="y1b")
            rb = tp.tile([P, Kreal], f32, name="rb")
            mb = tp.tile([P, Kreal], mybir.dt.uint8, name="mb")
            y16 = tp.tile([P, KMAX], f16, name="y16")

            for g in range(NCH):
                nc.sync.dma_start(t_seed[g][:], d_seed[g][:])
                nc.sync.dma_start(t_idx2[g][:], d_idx2[g][:])
            nc.sync.dma_start(t_rmask[:], d_rmask[:])
            for ci in range(NR1):
                nc.sync.dma_start(t_idx1[ci][:], d_idx1[ci][:])
            nc.sync.dma_start(t_wexp[:], d_wexp[:])
            nc.sync.dma_start(t_bin[:], d_bin[:])
            make_identity(nc, ident[:])
            nc.sync.dma_start(y2d[:], d_yin[:].rearrange("o (p q) -> (o p) q", p=P))
            nc.vector.memset(y16[:], 0.0)

            r1_by_g = {}
            for ci, (g2, t0, t1) in enumerate(meta["r1_struct"]):
                r1_by_g.setdefault(g2, []).append((ci, t0, t1))

            # small chunk last: its short r1->copy->r2 chain ends the iteration
            g_order = [1, 0, 2] if NCH == 3 else list(range(NCH))

            def body(last=False):
                nbatch = 0
                for g in g_order:
                    w0, w1 = int(EB[g]), int(EB[g + 1])
                    mw = int(M1[g])
                    # seed run-starts for chunk g, then segmented forward-fill
                    nc.gpsimd.local_scatter(
                        seedb[g][:], y2d[:], t_seed[g][:],
                        channels=P, num_elems=mw, num_idxs=SD)
                    nc.vector.tensor_tensor_scan(
                        expb[g][:], t_rmask[:, w0:w1], seedb[g][:], 0.0,
                        op0=AOP.mult, op1=AOP.add)
                    nc.vector.tensor_tensor(expb[g][:], expb[g][:],
                                            t_wexp[:, w0:w1], op=AOP.mult)
                    # round 1 into per-chunk staging
                    for ci, t0, t1 in r1_by_g[g]:
                        nt = t1 - t0
                        nc.gpsimd.local_scatter(
                            stag[g][:, 128 * t0:128 * t1], expb[g][:],
                            t_idx1[ci][:], channels=P, num_elems=128 * nt,
                            num_idxs=mw)
                    # transposes; PSUM->SBUF copies alternate Act/DVE
                    Tg = int(MTg[g])
                    for tb0 in range(0, Tg, 8):
                        nb = min(8, Tg - tb0)
                        pt = pp.tile([P, 8 * P], f16, space="PSUM", tag="tr",
                                     name="tr")
                        for t in range(tb0, tb0 + nb):
                            nc.tensor.transpose(
                                pt[:, 128 * (t - tb0):128 * (t - tb0 + 1)],
                                stag[g][:, 128 * t:128 * (t + 1)], ident[:])
                        dst = t2d[g][:, 128 * tb0:128 * (tb0 + nb)]
                        if nbatch % 2 == 0:
                            nc.vector.tensor_copy(dst, pt[:, 0:128 * nb])
                        else:
                            nc.scalar.copy(dst, pt[:, 0:128 * nb])
                        nbatch += 1
                    # round 2 into dest slots
                    nc.gpsimd.local_scatter(
                        slots[g][:].rearrange("p k s -> p (k s)"), t2d[g][:],
                        t_idx2[g][:], channels=P, num_elems=DSTW[g],
                        num_idxs=128 * Tg)
                    # segmented reduce for chunk g (fp16 out: values |w*y|<0.2,
                    # 32-wide sums stay O(1); validated vs fp64 reference)
                    c0 = g * CH // 32
                    with nc.allow_low_precision(reason="32-wide fp16 slot sums"):
                        nc.vector.tensor_reduce(
                            sp[:, c0:c0 + DSTW[g] // 32], slots[g][:],
                            axis=mybir.AxisListType.X, op=AOP.add)
                for r in (2, 3, 4):
                    nr = meta["nr_max"][r]
                    if nr:
                        b0 = meta["region_base"][r]
                        nc.vector.tensor_tensor(sp[:, 0:nr], sp[:, 0:nr],
                                                sp[:, b0:b0 + nr], op=AOP.add)
                nc.vector.tensor_tensor(vv[:], sp[:, 0:Kreal], t_bin[:], op=AOP.add)
                nc.vector.scalar_tensor_tensor(
                    y1b[:], vv[:], float(LEAK), vv[:], op0=AOP.mult, op1=AOP.max)
                nc.vector.tensor_scalar_max(rb[:], vv[:], 0.5)
                nc.vector.reciprocal(rb[:], rb[:])
                nc.vector.tensor_scalar(rb[:], rb[:], -0.25, 1.0,
                                        op0=AOP.mult, op1=AOP.add)
                nc.vector.tensor_scalar(mb[:], vv[:], 0.5, None, op0=AOP.is_gt)
                nc.vector.select(y16[:, 0:Kreal], mb[:], rb[:], y1b[:])
                if last:
                    return  # final shard never leaves this core pre-gather
                nc.sync.dma_start(
                    d_ysh[:].rearrange("o (p k) -> (o p) k", p=P), y16[:])
                if not no_cc:
                    nc.gpsimd.collective_compute(
                        "AllGather", AOP.bypass,
                        replica_groups=[list(range(NCORES))],
                        ins=[d_ysh[:]], outs=[d_yfull[:]])
                nc.sync.dma_start(
                    y2d[:], d_yfull[:].rearrange("o (p q) -> (o p) q", p=P))

            for it in range(n_iters):
                body(last=(skip_last_exchange and it == n_iters - 1))
            nc.sync.dma_start(d_yout[:], y16[:, 0:Kreal])
            nc.sync.dma_start(
                d_yall[:].rearrange("o (p q) -> (o p) q", p=P), y2d[:])

    nc.compile()
    return nc


def _in_maps(cores, meta):
    maps = []
    for tb in cores:
        m = {"t_wexp": tb["w_exp"], "t_bin": tb["b_in_t"],
             "t_rmask": tb["runmask"]}
        for g in range(meta["NCH"]):
            m[f"t_seed{g}"] = tb["seedidx"][g]
            m[f"t_idx2_{g}"] = tb["idx2"][g]
        for ci in range(meta["NR1"]):
            m[f"t_idx1_{ci}"] = tb["idx1"][ci]
        maps.append(m)
    return maps


def _gather_y(res, meta):
    Kreal = meta["Kreal"]
    y_full = np.zeros(NC_PAD, np.float32)
    jj, kk2 = np.meshgrid(np.arange(P), np.arange(Kreal), indexing="ij")
    for c in range(NCORES):
        y32 = res.results[c]["y_out"]
        y_full[SHARD * c + KMAX * jj.ravel() + kk2.ravel()] = y32.ravel()
    return y_full


SEG = 150  # whole run fits one NEFF


def kernel(**inputs):
    from concourse.bass_utils import run_bass_kernel_spmd
    inputs = {k: np.asarray(v) for k, v in inputs.items()}
    cores, perm, meta = _prep(**inputs)
    nseg = _ceil(ITERS, SEG)
    nc = _build(cores, meta, SEG, skip_last_exchange=(nseg == 1))
    maps = _in_maps(cores, meta)
    y_state = np.zeros((1, NC_PAD), np.float16)
    res = None
    for s in range(nseg):
        for m in maps:
            m["y_in"] = y_state
        res = run_bass_kernel_spmd(nc, [dict(m) for m in maps],
                                   core_ids=list(range(NCORES)))
        y_state = res.results[0]["y_all"]
    y_old = _gather_y(res, meta)[perm]
    out = (inputs["out_weights"].astype(np.float32)
           * y_old[inputs["out_indices"]])[None, :]
    return out.astype(np.float32)


if __name__ == "__main__":
    import sys, time
    sys.path.insert(0, "/root/problem")
    import reference
    inputs = {k: np.asarray(v) for k, v in reference.setup_inputs().items()}
    t0 = time.time()
    cores, perm, meta = _prep(**inputs)
    print(f"prep {time.time()-t0:.1f}s Kreal={meta['Kreal']} KP={meta['KP']} "
          f"FD={meta['FD']} M1={meta['M1']} MTg={meta['MTg']} T={meta['T']} "
          f"MEXP={meta['MEXP']} NR1={meta['NR1']}")
    if "sim" in sys.argv:
        n_it = int(sys.argv[sys.argv.index("sim") + 1]) if len(sys.argv) > 2 else 8
        import jax.numpy as jnp
        ni = np.asarray(jnp.zeros((N,), jnp.float32).at[jnp.asarray(inputs["in_indices"])].set(
            jnp.asarray(inputs["in_weights"], jnp.float32) * jnp.asarray(inputs["x"][0], jnp.float32)))
        b_in = (ni + inputs["biases"]).astype(np.float64)
        rw = inputs["rec_weights"].astype(np.float64)
        er, ec = inputs["edge_rows"], inputs["edge_cols"]
        yref = np.zeros(N, np.float64)
        for _ in range(n_it):
            s = np.bincount(er, weights=rw * yref[ec], minlength=N)
            v = s + b_in
            yref = np.where(v > 0.5, 1.0 - 0.25 / np.maximum(v, 0.5),
                            np.maximum(v, LEAK * v))
        scale = np.abs(yref).max()
        t0 = time.time()
        ys = _sim(cores, perm, meta, n_it, quant=False)
        print(f"sim(noquant,{n_it}) {time.time()-t0:.1f}s  max rel err:",
              np.abs(ys[perm] - yref).max() / scale)
        t0 = time.time()
        ysq = _sim(cores, perm, meta, n_it, quant=True)
        print(f"sim(fp16,{n_it}) {time.time()-t0:.1f}s  max rel err:",
              np.abs(ysq[perm] - yref).max() / scale)



# revision 27
# speedup vs baseline: 1.2554x; 1.0143x over previous
"""Bionetwork sparse-matvec recurrence on 8 trn2 NeuronCores.

y_{t+1} = act(A y_t + b_in), 150 iterations, A fixed sparse (3.2M edges,
100k nodes).  Dest-sharded across 8 cores; all routing tables SBUF-resident.

Per iteration, per core (local_scatter = vectorized GPSIMD within-row scatter):
  1. seed-scatter per dest-chunk g: canonical y -> run-starts of expansion
  2. segmented forward-fill via one tensor_tensor_scan (state=mask*state+seed)
  3. multiply by edge weights (fp16, in place)
  4. round-1 local_scatter: products -> staging tiles at col 128*t + dest_row
  5. PE transpose of each [128,128] staging tile (the cross-partition hop)
  6. round-2 local_scatter: transposed stream -> dest-slot layout
  7. segmented reduce (32-wide slots, fp16); fold pseudo-slot regions
  8. v = s + b_in; piecewise activation (select-fused); AllGather; reload y

Everything is table-driven; tables are built host-side from the (fixed)
edge lists and shipped as per-core input tensors to one shared program.
"""
import numpy as np

N = 100000
E = 3200000
P = 128
NCORES = 8
QW = 800                    # canonical width: 128*800 = 102400
NC_PAD = P * QW
SHARD = NC_PAD // NCORES    # 12800 = 128*100
KMAX = SHARD // P           # 100
ITERS = 150
LEAK = 0.01
RUN_CAP = 16                # fill rounds 1,2,4,8 cover runs of 16
SEED_REGIONS = 1
MAX_DST = 2046
TILES_PER_CALL = 15
SD = SEED_REGIONS * QW


def _ceil(a, b):
    return -(-a // b)


def _prep(x, in_weights, rec_weights, biases, out_weights,
          in_indices, edge_rows, edge_cols, out_indices):
    deg = np.bincount(edge_rows, minlength=N)
    npseudo = np.maximum(1, _ceil(deg, 32))
    assert npseudo.max() <= 4, f"max in-degree {deg.max()} > 128 unsupported"

    # deal dests round-robin over 1024 (core,row) bins; sort by npseudo desc
    # (region contiguity) but shuffle within classes (chunk load balance).
    # The within-class shuffle seed sets the max edges per (chunk, src
    # partition, dest row) cell, which sets the staging-tile count T and with
    # it the round-1/round-2 scatter cost -- pick the best of several seeds
    # by an exact vectorized estimate of T.
    i = np.arange(N)
    b = i % (NCORES * P)
    core_of, row_of, k_of = b % NCORES, b // NCORES, i // (NCORES * P)
    Kreal = int(k_of.max()) + 1
    assert Kreal <= KMAX
    rb_est = {1: 0, 2: Kreal, 3: Kreal, 4: Kreal}
    nr2 = _ceil(int((npseudo >= 2).sum()), NCORES * P)
    KP_est = Kreal + nr2
    FD_est = 32 * KP_est
    NCH_e = _ceil(FD_est, MAX_DST)
    CH_est = _ceil(_ceil(FD_est, NCH_e), 32) * 32
    NCH_e = _ceil(FD_est, CH_est)

    def _deal(seed):
        rng = np.random.default_rng(seed)
        order = np.lexsort((rng.permutation(N), -npseudo))
        pm = np.empty(N, np.int64)
        pm[order] = SHARD * core_of + KMAX * row_of + k_of
        return pm

    def _t_est(pm):
        dn, sn = pm[edge_rows], pm[edge_cols]
        so = np.argsort(dn, kind="stable")
        ds = dn[so]
        st = np.r_[0, np.flatnonzero(np.diff(ds)) + 1]
        sid = np.zeros(dn.size, np.int64)
        sid[st[1:]] = 1
        sid = np.cumsum(sid)
        rank = np.arange(dn.size) - st[sid]
        rnk = np.empty(dn.size, np.int64)
        rnk[so] = rank
        cd = dn // SHARD
        loc = dn - SHARD * cd
        jd, kd = loc // KMAX, loc % KMAX
        r_idx = np.minimum(rnk // 32, 1)
        f = 32 * (np.where(r_idx == 0, kd, Kreal + kd)) + rnk % 32
        g = f // CH_est
        key = ((cd * NCH_e + g) * P + sn // QW) * P + jd
        cnt = np.bincount(key, minlength=NCORES * NCH_e * P * P)
        cg = cnt.reshape(NCORES, NCH_e, P * P).max(axis=2).max(axis=0)
        return int(cg.sum()), cg

    best = None
    for seed in (12345, 1, 7, 42, 2026):
        pm = _deal(seed)
        te, cg = _t_est(pm)
        if best is None or te < best[0]:
            best = (te, seed, pm)
    perm = best[2]

    nr_max = {r: _ceil(int((npseudo >= r).sum()), NCORES * P) for r in (2, 3, 4)}
    region_base = {1: 0}
    base = Kreal
    for r in (2, 3, 4):
        region_base[r] = base
        base += nr_max[r]
    KP = base
    FD = 32 * KP
    NCH = _ceil(FD, MAX_DST)
    CH = _ceil(_ceil(FD, NCH), 32) * 32
    NCH = _ceil(FD, CH)

    import jax.numpy as jnp
    node_in = np.asarray(
        jnp.zeros((N,), jnp.float32).at[jnp.asarray(in_indices)].set(
            jnp.asarray(in_weights, jnp.float32) * jnp.asarray(x[0], jnp.float32)))
    b_in_full = node_in + biases.astype(np.float32)

    dnew, snew = perm[edge_rows], perm[edge_cols]
    w_all = rec_weights.astype(np.float32)
    dcore = dnew // SHARD

    # ---------- pass 1: per-core edge geometry ----------
    geo = []
    for c in range(NCORES):
        em = np.where(dcore == c)[0]
        d_loc = dnew[em] - SHARD * c
        j, k = d_loc // KMAX, d_loc % KMAX
        s_new = snew[em]
        p0, q0 = s_new // QW, s_new % QW
        w = w_all[em]
        ne = em.size

        def ranks_of(key):
            so = np.argsort(key, kind="stable")
            ks = key[so]
            st = np.r_[0, np.flatnonzero(np.diff(ks)) + 1]
            sid = np.zeros(ne, np.int64)
            sid[st[1:]] = 1
            sid = np.cumsum(sid)
            r = np.arange(ne) - st[sid]
            out = np.empty(ne, np.int64)
            out[so] = r
            return out

        slot = ranks_of(d_loc)
        r_idx = slot // 32
        rbv = np.array([region_base[1], region_base[2], region_base[3], region_base[4]])
        f = 32 * (rbv[r_idx] + k) + slot % 32
        g = f // CH
        trank = ranks_of((g * P + p0) * P + j)
        # expansion position within (g,p0) ordered by q0, and rank within source
        so3 = np.lexsort((q0, p0, g))
        gp = (g * P + p0)[so3]
        st = np.r_[0, np.flatnonzero(np.diff(gp)) + 1]
        sid = np.zeros(ne, np.int64)
        sid[st[1:]] = 1
        sid = np.cumsum(sid)
        m_pos = np.empty(ne, np.int64)
        m_pos[so3] = np.arange(ne) - st[sid]
        gpq = ((g * P + p0) * QW + q0)[so3]
        st4 = np.r_[0, np.flatnonzero(np.diff(gpq)) + 1]
        sid4 = np.zeros(ne, np.int64)
        sid4[st4[1:]] = 1
        sid4 = np.cumsum(sid4)
        src_rank = np.empty(ne, np.int64)
        src_rank[so3] = np.arange(ne) - st4[sid4]
        geo.append(dict(j=j, p0=p0, q0=q0, w=w, f=f, g=g,
                        trank=trank, m_pos=m_pos, src_rank=src_rank, ne=ne))

    # uniform per-chunk sizes across cores
    M1 = np.zeros(NCH, np.int64)
    MTg = np.zeros(NCH, np.int64)
    for gg in geo:
        for g2 in range(NCH):
            sel = gg["g"] == g2
            if sel.any():
                M1[g2] = max(M1[g2], int(gg["m_pos"][sel].max()) + 1)
                MTg[g2] = max(MTg[g2], int(gg["trank"][sel].max()) + 1)
    M1 = (_ceil(M1, 2) * 2).astype(np.int64)
    EB = np.r_[0, np.cumsum(M1)]         # expansion bases
    MEXP = int(EB[-1])
    TBASE = np.r_[0, np.cumsum(MTg)]     # tile bases
    T = int(TBASE[-1])
    # round-1 call structure: (g, t0, t1), evenly-split windows <= 15 tiles
    r1_struct = []
    for g2 in range(NCH):
        tg = int(MTg[g2])
        ncall = _ceil(tg, TILES_PER_CALL)
        base, rem = divmod(tg, ncall)
        t0 = 0
        for ci in range(ncall):
            nt = base + (1 if ci < rem else 0)
            r1_struct.append((g2, t0, t0 + nt))
            t0 += nt
    NR1 = len(r1_struct)

    # ---------- pass 2: tables ----------
    cores = []
    for c in range(NCORES):
        gg = geo[c]
        j, p0, q0, w = gg["j"], gg["p0"], gg["q0"], gg["w"]
        f, g, trank, m_pos, src_rank = (gg["f"], gg["g"], gg["trank"],
                                        gg["m_pos"], gg["src_rank"])
        m_glob = EB[g] + m_pos
        dist = src_rank

        seedidx = np.full((NCH, P, SD), -1, np.int16)
        sm = dist == 0
        seedidx[g[sm], p0[sm], q0[sm]] = m_pos[sm].astype(np.int16)

        # scan fill mask: 1.0 inside a source run (copy state), 0.0 at starts
        runmask = np.zeros((P, MEXP), np.float16)
        mm = dist > 0
        runmask[p0[mm], m_glob[mm]] = 1.0

        w_exp = np.zeros((P, MEXP), np.float16)
        w_exp[p0, m_glob] = w.astype(np.float16)

        idx1 = []
        for (g2, t0, t1) in r1_struct:
            sel = (g == g2) & (trank >= t0) & (trank < t1)
            idx = np.full((P, int(M1[g2])), -1, np.int16)
            idx[p0[sel], m_pos[sel]] = (128 * (trank[sel] - t0) + j[sel]).astype(np.int16)
            idx1.append(idx)

        idx2 = []
        for g2 in range(NCH):
            sel = g == g2
            idx = np.full((P, 128 * int(MTg[g2])), -1, np.int16)
            idx[j[sel], 128 * trank[sel] + p0[sel]] = (f[sel] - g2 * CH).astype(np.int16)
            idx2.append(idx)

        b_in_t = np.zeros((P, Kreal), np.float32)
        nid = np.where((perm >= SHARD * c) & (perm < SHARD * (c + 1)))[0]
        dl = perm[nid] - SHARD * c
        b_in_t[dl // KMAX, dl % KMAX] = b_in_full[nid]

        cores.append(dict(seedidx=seedidx, runmask=runmask, w_exp=w_exp,
                          idx1=idx1, idx2=idx2, b_in_t=b_in_t))

    meta = dict(Kreal=Kreal, KP=KP, FD=FD, NCH=NCH, CH=CH, M1=M1, EB=EB,
                MTg=MTg, TBASE=TBASE, T=T, MEXP=MEXP, NR1=NR1,
                r1_struct=r1_struct, nr_max=nr_max, region_base=region_base)
    return cores, perm, meta


def _act_np(v):
    y1 = np.maximum(v, np.float32(LEAK) * v)
    ysat = (1.0 - 0.25 / np.maximum(v, 0.5)).astype(v.dtype)
    return np.where(v > 0.5, ysat, y1)


def _sim(cores, perm, meta, n_iters, quant=True):
    dt = np.float16 if quant else np.float32
    Kreal, KP, FD, NCH, CH = (meta["Kreal"], meta["KP"], meta["FD"],
                              meta["NCH"], meta["CH"])
    M1, EB, MTg, TBASE, T, MEXP = (meta["M1"], meta["EB"], meta["MTg"],
                                   meta["TBASE"], meta["T"], meta["MEXP"])
    y = np.zeros(NC_PAD, np.float32)
    for it in range(n_iters):
        y2d = y.reshape(P, QW).astype(dt)
        seed_data = y2d
        y_next = np.zeros(NC_PAD, np.float32)
        for c, tb in enumerate(cores):
            seeds = np.zeros((P, MEXP), dt)
            for g2 in range(NCH):
                sidx = tb["seedidx"][g2]
                pp, cc = np.where(sidx >= 0)
                seeds[pp, EB[g2] + sidx[pp, cc]] = seed_data[pp, cc]
            # segmented forward-fill scan: state = mask*state + seed (fp32
            # state, downcast per element) per chunk
            exp_t = np.zeros((P, MEXP), dt)
            rm = tb["runmask"].astype(np.float32)
            sd32 = seeds.astype(np.float32)
            for g2 in range(NCH):
                st = np.zeros(P, np.float32)
                for t in range(int(EB[g2]), int(EB[g2 + 1])):
                    st = rm[:, t] * st + sd32[:, t]
                    exp_t[:, t] = st.astype(dt)
            prod = (exp_t.astype(np.float32) * tb["w_exp"].astype(np.float32)).astype(dt)
            staging = np.zeros((P, 128 * T), dt)
            for ci, (g2, t0, t1) in enumerate(meta["r1_struct"]):
                idx = tb["idx1"][ci]
                data = prod[:, EB[g2]:EB[g2] + M1[g2]]
                pp, cc = np.where(idx >= 0)
                staging[pp, 128 * (TBASE[g2] + t0) + idx[pp, cc]] = data[pp, cc]
            t2 = np.zeros_like(staging)
            for t in range(T):
                t2[:, 128 * t:128 * (t + 1)] = staging[:, 128 * t:128 * (t + 1)].T
            slots = np.zeros((P, FD), dt)
            for g2 in range(NCH):
                idx = tb["idx2"][g2]
                data = t2[:, 128 * TBASE[g2]:128 * (TBASE[g2] + MTg[g2])]
                pp, cc = np.where(idx >= 0)
                slots[pp, g2 * CH + idx[pp, cc]] = data[pp, cc]
            sp = slots.reshape(P, KP, 32).astype(np.float32).sum(axis=2).astype(dt).astype(np.float32)
            s = sp[:, :Kreal].copy()
            for r in (2, 3, 4):
                nr = meta["nr_max"][r]
                if nr:
                    b0 = meta["region_base"][r]
                    s[:, :nr] += sp[:, b0:b0 + nr]
            v = s + tb["b_in_t"]
            y32 = _act_np(v)
            jj, kk2 = np.meshgrid(np.arange(P), np.arange(Kreal), indexing="ij")
            y_next[SHARD * c + KMAX * jj.ravel() + kk2.ravel()] = y32.ravel()
        y = y_next
    return y


# ============================ BASS KERNEL ============================

def _build(cores, meta, n_iters, no_cc=False, skip_last_exchange=False):
    import concourse.bacc as bacc
    import concourse.mybir as mybir
    import concourse.tile as tile
    from concourse.masks import make_identity

    f16, f32, i16 = mybir.dt.float16, mybir.dt.float32, mybir.dt.int16
    AOP = mybir.AluOpType
    Kreal, KP, FD, NCH, CH = (meta["Kreal"], meta["KP"], meta["FD"],
                              meta["NCH"], meta["CH"])
    M1, EB, MTg, TBASE, T, MEXP, NR1 = (meta["M1"], meta["EB"], meta["MTg"],
                                        meta["TBASE"], meta["T"],
                                        meta["MEXP"], meta["NR1"])
    DSTW = [min(FD, (g + 1) * CH) - g * CH for g in range(NCH)]

    nc = bacc.Bacc("TRN2", target_bir_lowering=False)

    d_seed = [nc.dram_tensor(f"t_seed{g}", [P, SD], i16, kind="ExternalInput")
              for g in range(NCH)]
    d_rmask = nc.dram_tensor("t_rmask", [P, MEXP], f16, kind="ExternalInput")
    d_wexp = nc.dram_tensor("t_wexp", [P, MEXP], f16, kind="ExternalInput")
    d_idx1 = [nc.dram_tensor(f"t_idx1_{ci}", [P, int(M1[g2])], i16,
                             kind="ExternalInput")
              for ci, (g2, _, _) in enumerate(meta["r1_struct"])]
    d_idx2 = [nc.dram_tensor(f"t_idx2_{g}", [P, 128 * int(MTg[g])], i16,
                             kind="ExternalInput") for g in range(NCH)]
    d_bin = nc.dram_tensor("t_bin", [P, Kreal], f32, kind="ExternalInput")
    d_yout = nc.dram_tensor("y_out", [P, Kreal], f16, kind="ExternalOutput")
    d_ysh = nc.dram_tensor("y_shard", [1, SHARD], f16, kind="Internal")
    d_yfull = nc.dram_tensor("y_full", [1, NC_PAD], f16, kind="Internal",
                             addr_space="Shared")
    d_yin = nc.dram_tensor("y_in", [1, NC_PAD], f16, kind="ExternalInput")
    d_yall = nc.dram_tensor("y_all", [1, NC_PAD], f16, kind="ExternalOutput")

    with tile.TileContext(nc) as tc:
        with tc.tile_pool(name="tables", bufs=1) as tp, \
             tc.tile_pool(name="psum", bufs=8, space="PSUM") as pp:
            t_seed = [tp.tile([P, SD], i16, name=f"seed{g}") for g in range(NCH)]
            t_rmask = tp.tile([P, MEXP], f16, name="rmask")
            t_wexp = tp.tile([P, MEXP], f16, name="wexp")
            t_idx1 = [tp.tile([P, int(M1[g2])], i16, name=f"i1_{ci}")
                      for ci, (g2, _, _) in enumerate(meta["r1_struct"])]
            t_idx2 = [tp.tile([P, 128 * int(MTg[g])], i16, name=f"i2_{g}")
                      for g in range(NCH)]
            t_bin = tp.tile([P, Kreal], f32, name="bin")
            ident = tp.tile([P, P], f16, name="ident")
            y2d = tp.tile([P, QW], f16, name="y2d")
            expb = [tp.tile([P, int(M1[g])], f16, name=f"expb{g}")
                    for g in range(NCH)]
            seedb = [tp.tile([P, int(M1[g])], f16, name=f"seedb{g}")
                     for g in range(NCH)]
            stag = [tp.tile([P, 128 * int(MTg[g])], f16, name=f"stag{g}")
                    for g in range(NCH)]
            t2d = [tp.tile([P, 128 * int(MTg[g])], f16, name=f"t2d{g}")
                   for g in range(NCH)]
            slots = [tp.tile([P, DSTW[g] // 32, 32], f16, name=f"slots{g}")
                     for g in range(NCH)]
            sp = tp.tile([P, KP], f16, name="sp")
            vv = tp.tile([P, Kreal], f32, name="vv")
            y1b = tp.tile([P, Kreal], f32, name="y1b")
            rb = tp.tile([P, Kreal], f32, name="rb")
            mb = tp.tile([P, Kreal], mybir.dt.uint8, name="mb")
            y16 = tp.tile([P, KMAX], f16, name="y16")

            for g in range(NCH):
                nc.sync.dma_start(t_seed[g][:], d_seed[g][:])
                nc.sync.dma_start(t_idx2[g][:], d_idx2[g][:])
            nc.sync.dma_start(t_rmask[:], d_rmask[:])
            for ci in range(NR1):
                nc.sync.dma_start(t_idx1[ci][:], d_idx1[ci][:])
            nc.sync.dma_start(t_wexp[:], d_wexp[:])
            nc.sync.dma_start(t_bin[:], d_bin[:])
            make_identity(nc, ident[:])
            nc.sync.dma_start(y2d[:], d_yin[:].rearrange("o (p q) -> (o p) q", p=P))
            nc.vector.memset(y16[:], 0.0)

            r1_by_g = {}
            for ci, (g2, t0, t1) in enumerate(meta["r1_struct"]):
                r1_by_g.setdefault(g2, []).append((ci, t0, t1))

            # small chunk last: its short r1->copy->r2 chain ends the iteration
            g_order = [1, 0, 2] if NCH == 3 else list(range(NCH))

            def body(last=False):
                nbatch = 0
                for g in g_order:
                    w0, w1 = int(EB[g]), int(EB[g + 1])
                    mw = int(M1[g])
                    # seed run-starts for chunk g, then segmented forward-fill
                    nc.gpsimd.local_scatter(
                        seedb[g][:], y2d[:], t_seed[g][:],
                        channels=P, num_elems=mw, num_idxs=SD)
                    nc.vector.tensor_tensor_scan(
                        expb[g][:], t_rmask[:, w0:w1], seedb[g][:], 0.0,
                        op0=AOP.mult, op1=AOP.add)
                    nc.vector.tensor_tensor(expb[g][:], expb[g][:],
                                            t_wexp[:, w0:w1], op=AOP.mult)
                    # round 1 into per-chunk staging
                    for ci, t0, t1 in r1_by_g[g]:
                        nt = t1 - t0
                        nc.gpsimd.local_scatter(
                            stag[g][:, 128 * t0:128 * t1], expb[g][:],
                            t_idx1[ci][:], channels=P, num_elems=128 * nt,
                            num_idxs=mw)
                    # transposes; PSUM->SBUF copies alternate Act/DVE
                    Tg = int(MTg[g])
                    for tb0 in range(0, Tg, 8):
                        nb = min(8, Tg - tb0)
                        pt = pp.tile([P, 8 * P], f16, space="PSUM", tag="tr",
                                     name="tr")
                        for t in range(tb0, tb0 + nb):
                            nc.tensor.transpose(
                                pt[:, 128 * (t - tb0):128 * (t - tb0 + 1)],
                                stag[g][:, 128 * t:128 * (t + 1)], ident[:])
                        dst = t2d[g][:, 128 * tb0:128 * (tb0 + nb)]
                        if nbatch % 2 == 0:
                            nc.vector.tensor_copy(dst, pt[:, 0:128 * nb])
                        else:
                            nc.scalar.copy(dst, pt[:, 0:128 * nb])
                        nbatch += 1
                    # round 2 into dest slots
                    nc.gpsimd.local_scatter(
                        slots[g][:].rearrange("p k s -> p (k s)"), t2d[g][:],
                        t_idx2[g][:], channels=P, num_elems=DSTW[g],
                        num_idxs=128 * Tg)
                    # segmented reduce for chunk g (fp16 out: values |w*y|<0.2,
                    # 32-wide sums stay O(1); validated vs fp64 reference)
                    c0 = g * CH // 32
                    with nc.allow_low_precision(reason="32-wide fp16 slot sums"):
                        nc.vector.tensor_reduce(
                            sp[:, c0:c0 + DSTW[g] // 32], slots[g][:],
                            axis=mybir.AxisListType.X, op=AOP.add)
                for r in (2, 3, 4):
                    nr = meta["nr_max"][r]
                    if nr:
                        b0 = meta["region_base"][r]
                        nc.vector.tensor_tensor(sp[:, 0:nr], sp[:, 0:nr],
                                                sp[:, b0:b0 + nr], op=AOP.add)
                nc.vector.tensor_tensor(vv[:], sp[:, 0:Kreal], t_bin[:], op=AOP.add)
                nc.vector.scalar_tensor_tensor(
                    y1b[:], vv[:], float(LEAK), vv[:], op0=AOP.mult, op1=AOP.max)
                nc.vector.tensor_scalar_max(rb[:], vv[:], 0.5)
                nc.vector.reciprocal(rb[:], rb[:])
                nc.vector.tensor_scalar(rb[:], rb[:], -0.25, 1.0,
                                        op0=AOP.mult, op1=AOP.add)
                nc.vector.tensor_scalar(mb[:], vv[:], 0.5, None, op0=AOP.is_gt)
                nc.vector.select(y16[:, 0:Kreal], mb[:], rb[:], y1b[:])
                if last:
                    return  # final shard never leaves this core pre-gather
                nc.sync.dma_start(
                    d_ysh[:].rearrange("o (p k) -> (o p) k", p=P), y16[:])
                if not no_cc:
                    nc.gpsimd.collective_compute(
                        "AllGather", AOP.bypass,
                        replica_groups=[list(range(NCORES))],
                        ins=[d_ysh[:]], outs=[d_yfull[:]])
                nc.sync.dma_start(
                    y2d[:], d_yfull[:].rearrange("o (p q) -> (o p) q", p=P))

            for it in range(n_iters):
                body(last=(skip_last_exchange and it == n_iters - 1))
            nc.sync.dma_start(d_yout[:], y16[:, 0:Kreal])
            nc.sync.dma_start(
                d_yall[:].rearrange("o (p q) -> (o p) q", p=P), y2d[:])

    nc.compile()
    return nc


def _in_maps(cores, meta):
    maps = []
    for tb in cores:
        m = {"t_wexp": tb["w_exp"], "t_bin": tb["b_in_t"],
             "t_rmask": tb["runmask"]}
        for g in range(meta["NCH"]):
            m[f"t_seed{g}"] = tb["seedidx"][g]
            m[f"t_idx2_{g}"] = tb["idx2"][g]
        for ci in range(meta["NR1"]):
            m[f"t_idx1_{ci}"] = tb["idx1"][ci]
        maps.append(m)
    return maps


def _gather_y(res, meta):
    Kreal = meta["Kreal"]
    y_full = np.zeros(NC_PAD, np.float32)
    jj, kk2 = np.meshgrid(np.arange(P), np.arange(Kreal), indexing="ij")
    for c in range(NCORES):
        y32 = res.results[c]["y_out"]
        y_full[SHARD * c + KMAX * jj.ravel() + kk2.ravel()] = y32.ravel()
    return y_full


SEG = 150  # whole run fits one NEFF


def kernel(**inputs):
    from concourse.bass_utils import run_bass_kernel_spmd
    inputs = {k: np.asarray(v) for k, v in inputs.items()}
    cores, perm, meta = _prep(**inputs)
    nseg = _ceil(ITERS, SEG)
    nc = _build(cores, meta, SEG, skip_last_exchange=(nseg == 1))
    maps = _in_maps(cores, meta)
    y_state = np.zeros((1, NC_PAD), np.float16)
    res = None
    for s in range(nseg):
        for m in maps:
            m["y_in"] = y_state
        res = run_bass_kernel_spmd(nc, [dict(m) for m in maps],
                                   core_ids=list(range(NCORES)))
        y_state = res.results[0]["y_all"]
    y_old = _gather_y(res, meta)[perm]
    out = (inputs["out_weights"].astype(np.float32)
           * y_old[inputs["out_indices"]])[None, :]
    return out.astype(np.float32)


if __name__ == "__main__":
    import sys, time
    sys.path.insert(0, "/root/problem")
    import reference
    inputs = {k: np.asarray(v) for k, v in reference.setup_inputs().items()}
    t0 = time.time()
    cores, perm, meta = _prep(**inputs)
    print(f"prep {time.time()-t0:.1f}s Kreal={meta['Kreal']} KP={meta['KP']} "
          f"FD={meta['FD']} M1={meta['M1']} MTg={meta['MTg']} T={meta['T']} "
          f"MEXP={meta['MEXP']} NR1={meta['NR1']}")
    if "sim" in sys.argv:
        n_it = int(sys.argv[sys.argv.index("sim") + 1]) if len(sys.argv) > 2 else 8
        import jax.numpy as jnp
        ni = np.asarray(jnp.zeros((N,), jnp.float32).at[jnp.asarray(inputs["in_indices"])].set(
            jnp.asarray(inputs["in_weights"], jnp.float32) * jnp.asarray(inputs["x"][0], jnp.float32)))
        b_in = (ni + inputs["biases"]).astype(np.float64)
        rw = inputs["rec_weights"].astype(np.float64)
        er, ec = inputs["edge_rows"], inputs["edge_cols"]
        yref = np.zeros(N, np.float64)
        for _ in range(n_it):
            s = np.bincount(er, weights=rw * yref[ec], minlength=N)
            v = s + b_in
            yref = np.where(v > 0.5, 1.0 - 0.25 / np.maximum(v, 0.5),
                            np.maximum(v, LEAK * v))
        scale = np.abs(yref).max()
        t0 = time.time()
        ys = _sim(cores, perm, meta, n_it, quant=False)
        print(f"sim(noquant,{n_it}) {time.time()-t0:.1f}s  max rel err:",
              np.abs(ys[perm] - yref).max() / scale)
        t0 = time.time()
        ysq = _sim(cores, perm, meta, n_it, quant=True)
        print(f"sim(fp16,{n_it}) {time.time()-t0:.1f}s  max rel err:",
              np.abs(ysq[perm] - yref).max() / scale)



# revision 34
# speedup vs baseline: 1.3985x; 1.1140x over previous
"""Bionetwork sparse-matvec recurrence on 8 trn2 NeuronCores.

y_{t+1} = act(A y_t + b_in), 150 iterations, A fixed sparse (3.2M edges,
100k nodes).  Dest-sharded across 8 cores; all routing tables SBUF-resident.

Per iteration, per core (local_scatter = vectorized GPSIMD within-row scatter):
  1. seed-scatter per dest-chunk g: canonical y -> run-starts of expansion
  2. segmented forward-fill via one tensor_tensor_scan (state=mask*state+seed)
  3. multiply by edge weights (fp16, in place)
  4. round-1 local_scatter: products -> staging tiles at col 128*t + dest_row
  5. PE transpose of each [128,128] staging tile (the cross-partition hop)
  6. round-2 local_scatter: transposed stream -> dest-slot layout
  7. segmented reduce (32-wide slots, fp16); fold pseudo-slot regions
  8. v = s + b_in; piecewise activation (select-fused); AllGather; reload y

Everything is table-driven; tables are built host-side from the (fixed)
edge lists and shipped as per-core input tensors to one shared program.
"""
import numpy as np

N = 100000
E = 3200000
P = 128
NCORES = 8
QW = 800                    # canonical width: 128*800 = 102400
NC_PAD = P * QW
SHARD = NC_PAD // NCORES    # 12800 = 128*100
KMAX = SHARD // P           # 100
ITERS = 150
LEAK = 0.01
RUN_CAP = 16                # fill rounds 1,2,4,8 cover runs of 16
SEED_REGIONS = 1
MAX_DST = 2046
TILES_PER_CALL = 15
SD = SEED_REGIONS * QW


def _ceil(a, b):
    return -(-a // b)


def _prep(x, in_weights, rec_weights, biases, out_weights,
          in_indices, edge_rows, edge_cols, out_indices):
    deg = np.bincount(edge_rows, minlength=N)
    assert deg.max() <= 64, f"max in-degree {deg.max()} > 64 unsupported"
    np2 = deg > 32  # wide dests get a 64-col slot, the rest a 32-col slot

    # Slot layout: every chunk holds NP2C 64-wide slots + NP1C 32-wide slots
    # per (core,row) bin, so ANY dest can be placed in ANY chunk.  A greedy
    # min-max pass then assigns dests to chunks to flatten the edge count per
    # (chunk, src partition, dest row) cell -- that max sets the staging tile
    # count T and with it the round-1/round-2 scatter cost.
    NP2C, NP1C = 15, 18
    NCH = 3
    CH = NP2C * 64 + NP1C * 32          # 1536
    FD = NCH * CH
    SLOTC = NP2C + NP1C                 # sp slots per chunk
    Kreal = KP = NCH * SLOTC            # 99
    assert Kreal <= KMAX
    NB = NCORES * P

    # deal dests round-robin over bins, np2 class first
    rng = np.random.default_rng(12345)
    order = np.lexsort((rng.permutation(N), ~np2))
    i = np.arange(N)
    binid_pos = i % NB
    c_node = np.empty(N, np.int64)
    j_node = np.empty(N, np.int64)
    bin_node = np.empty(N, np.int64)
    c_node[order] = binid_pos % NCORES
    j_node[order] = binid_pos // NCORES
    bin_node[order] = binid_pos
    N2 = int(np2.sum())
    assert _ceil(N2, NB) <= NP2C * NCH and _ceil(N - N2, NB) <= NP1C * NCH
    rank_pos = np.empty(N, np.int64)
    rank_pos[order[:N2]] = np.arange(N2) // NB
    i1 = np.arange(N2, N)
    rank_pos[order[N2:]] = (i1 - N2 - ((i1 % NB) - N2) % NB) // NB
    r2max = int(rank_pos[order[:N2]].max()) + 1 if N2 else 0
    r1max = int(rank_pos[order[N2:]].max()) + 1

    # greedy chunk assignment, one round per (class, rank): each bin places
    # its rank-r dest into the chunk minimizing that bin-row's max cell
    p0_of_node = 16 * c_node + j_node // 8
    e_b = bin_node[edge_rows]
    e_p0 = p0_of_node[edge_cols]
    e_key = np.where(np2[edge_rows], 0, 256) + rank_pos[edge_rows]
    eo = np.argsort(e_key, kind="stable")
    e_key_s = e_key[eo]
    n_key = np.where(np2, 0, 256) + rank_pos
    no = np.argsort(n_key, kind="stable")
    n_key_s = n_key[no]

    cells = np.zeros((NCH, NB, P), np.int32)
    cnt2 = np.zeros((NCH, NB), np.int32)
    cnt1 = np.zeros((NCH, NB), np.int32)
    band_of = np.zeros(N, np.int64)
    kloc_of = np.zeros(N, np.int64)

    def _rounds(base_key, nmax, cnt, cap):
        for r in range(nmax):
            key = base_key + r
            na, nb_ = np.searchsorted(n_key_s, [key, key + 1])
            if na == nb_:
                continue
            nodes_r = no[na:nb_]
            ea, ebnd = np.searchsorted(e_key_s, [key, key + 1])
            H = np.zeros((NB, P), np.int32)
            if ea < ebnd:
                es = eo[ea:ebnd]
                np.add.at(H, (e_b[es], e_p0[es]), 1)
            cmax = (cells + H[None]).max(axis=2) * 64 + cnt
            cmax[cnt >= cap] = 1 << 30
            band = np.argmin(cmax, axis=0)
            bsel = bin_node[nodes_r]
            bb = band[bsel]
            band_of[nodes_r] = bb
            kloc_of[nodes_r] = cnt[bb, bsel]
            for ch in range(NCH):
                m = bsel[bb == ch]
                cells[ch, m] += H[m]
                cnt[ch, m] += 1

    _rounds(0, r2max, cnt2, NP2C)
    _rounds(256, r1max, cnt1, NP1C)

    k_node = band_of * SLOTC + np.where(np2, kloc_of, NP2C + kloc_of)
    perm = SHARD * c_node + KMAX * j_node + k_node

    # per-edge slot column: rank within dest (stable edge order)
    so = np.argsort(edge_rows, kind="stable")
    ds = edge_rows[so]
    st = np.r_[0, np.flatnonzero(np.diff(ds)) + 1]
    sid = np.zeros(E, np.int64)
    sid[st[1:]] = 1
    sid = np.cumsum(sid)
    e_drank = np.empty(E, np.int64)
    e_drank[so] = np.arange(E) - st[sid]
    e_np2 = np2[edge_rows]
    e_kloc = kloc_of[edge_rows]
    f_local = np.where(e_np2, 64 * e_kloc + e_drank,
                       NP2C * 64 + 32 * e_kloc + e_drank)
    fglob = band_of[edge_rows] * CH + f_local

    import jax.numpy as jnp
    node_in = np.asarray(
        jnp.zeros((N,), jnp.float32).at[jnp.asarray(in_indices)].set(
            jnp.asarray(in_weights, jnp.float32) * jnp.asarray(x[0], jnp.float32)))
    b_in_full = node_in + biases.astype(np.float32)

    dnew, snew = perm[edge_rows], perm[edge_cols]
    w_all = rec_weights.astype(np.float32)
    dcore = dnew // SHARD

    # ---------- pass 1: per-core edge geometry ----------
    geo = []
    for c in range(NCORES):
        em = np.where(dcore == c)[0]
        d_loc = dnew[em] - SHARD * c
        j, k = d_loc // KMAX, d_loc % KMAX
        s_new = snew[em]
        p0, q0 = s_new // QW, s_new % QW
        w = w_all[em]
        ne = em.size

        def ranks_of(key):
            so = np.argsort(key, kind="stable")
            ks = key[so]
            st = np.r_[0, np.flatnonzero(np.diff(ks)) + 1]
            sid = np.zeros(ne, np.int64)
            sid[st[1:]] = 1
            sid = np.cumsum(sid)
            r = np.arange(ne) - st[sid]
            out = np.empty(ne, np.int64)
            out[so] = r
            return out

        f = fglob[em]
        g = f // CH
        trank = ranks_of((g * P + p0) * P + j)
        # expansion position within (g,p0) ordered by q0, and rank within source
        so3 = np.lexsort((q0, p0, g))
        gp = (g * P + p0)[so3]
        st = np.r_[0, np.flatnonzero(np.diff(gp)) + 1]
        sid = np.zeros(ne, np.int64)
        sid[st[1:]] = 1
        sid = np.cumsum(sid)
        m_pos = np.empty(ne, np.int64)
        m_pos[so3] = np.arange(ne) - st[sid]
        gpq = ((g * P + p0) * QW + q0)[so3]
        st4 = np.r_[0, np.flatnonzero(np.diff(gpq)) + 1]
        sid4 = np.zeros(ne, np.int64)
        sid4[st4[1:]] = 1
        sid4 = np.cumsum(sid4)
        src_rank = np.empty(ne, np.int64)
        src_rank[so3] = np.arange(ne) - st4[sid4]
        geo.append(dict(j=j, p0=p0, q0=q0, w=w, f=f, g=g,
                        trank=trank, m_pos=m_pos, src_rank=src_rank, ne=ne))

    # uniform per-chunk sizes across cores
    M1 = np.zeros(NCH, np.int64)
    MTg = np.zeros(NCH, np.int64)
    for gg in geo:
        for g2 in range(NCH):
            sel = gg["g"] == g2
            if sel.any():
                M1[g2] = max(M1[g2], int(gg["m_pos"][sel].max()) + 1)
                MTg[g2] = max(MTg[g2], int(gg["trank"][sel].max()) + 1)
    M1 = (_ceil(M1, 2) * 2).astype(np.int64)
    EB = np.r_[0, np.cumsum(M1)]         # expansion bases
    MEXP = int(EB[-1])
    TBASE = np.r_[0, np.cumsum(MTg)]     # tile bases
    T = int(TBASE[-1])
    # round-1 call structure: (g, t0, t1), evenly-split windows <= 15 tiles
    r1_struct = []
    for g2 in range(NCH):
        tg = int(MTg[g2])
        ncall = _ceil(tg, TILES_PER_CALL)
        base, rem = divmod(tg, ncall)
        t0 = 0
        for ci in range(ncall):
            nt = base + (1 if ci < rem else 0)
            r1_struct.append((g2, t0, t0 + nt))
            t0 += nt
    NR1 = len(r1_struct)

    # ---------- pass 2: tables ----------
    cores = []
    for c in range(NCORES):
        gg = geo[c]
        j, p0, q0, w = gg["j"], gg["p0"], gg["q0"], gg["w"]
        f, g, trank, m_pos, src_rank = (gg["f"], gg["g"], gg["trank"],
                                        gg["m_pos"], gg["src_rank"])
        m_glob = EB[g] + m_pos
        dist = src_rank

        seedidx = np.full((NCH, P, SD), -1, np.int16)
        sm = dist == 0
        seedidx[g[sm], p0[sm], q0[sm]] = m_pos[sm].astype(np.int16)

        # scan fill mask: 1.0 inside a source run (copy state), 0.0 at starts
        runmask = np.zeros((P, MEXP), np.float16)
        mm = dist > 0
        runmask[p0[mm], m_glob[mm]] = 1.0

        w_exp = np.zeros((P, MEXP), np.float16)
        w_exp[p0, m_glob] = w.astype(np.float16)

        idx1 = []
        for (g2, t0, t1) in r1_struct:
            sel = (g == g2) & (trank >= t0) & (trank < t1)
            idx = np.full((P, int(M1[g2])), -1, np.int16)
            idx[p0[sel], m_pos[sel]] = (128 * (trank[sel] - t0) + j[sel]).astype(np.int16)
            idx1.append(idx)

        idx2 = []
        for g2 in range(NCH):
            sel = g == g2
            idx = np.full((P, 128 * int(MTg[g2])), -1, np.int16)
            idx[j[sel], 128 * trank[sel] + p0[sel]] = (f[sel] - g2 * CH).astype(np.int16)
            idx2.append(idx)

        b_in_t = np.zeros((P, Kreal), np.float32)
        nid = np.where((perm >= SHARD * c) & (perm < SHARD * (c + 1)))[0]
        dl = perm[nid] - SHARD * c
        b_in_t[dl // KMAX, dl % KMAX] = b_in_full[nid]

        cores.append(dict(seedidx=seedidx, runmask=runmask, w_exp=w_exp,
                          idx1=idx1, idx2=idx2, b_in_t=b_in_t))

    meta = dict(Kreal=Kreal, KP=KP, FD=FD, NCH=NCH, CH=CH, M1=M1, EB=EB,
                MTg=MTg, TBASE=TBASE, T=T, MEXP=MEXP, NR1=NR1,
                r1_struct=r1_struct, NP2C=NP2C, NP1C=NP1C, SLOTC=SLOTC)
    return cores, perm, meta


def _act_np(v):
    y1 = np.maximum(v, np.float32(LEAK) * v)
    ysat = (1.0 - 0.25 / np.maximum(v, 0.5)).astype(v.dtype)
    return np.where(v > 0.5, ysat, y1)


def _sim(cores, perm, meta, n_iters, quant=True):
    dt = np.float16 if quant else np.float32
    Kreal, KP, FD, NCH, CH = (meta["Kreal"], meta["KP"], meta["FD"],
                              meta["NCH"], meta["CH"])
    M1, EB, MTg, TBASE, T, MEXP = (meta["M1"], meta["EB"], meta["MTg"],
                                   meta["TBASE"], meta["T"], meta["MEXP"])
    y = np.zeros(NC_PAD, np.float32)
    for it in range(n_iters):
        y2d = y.reshape(P, QW).astype(dt)
        seed_data = y2d
        y_next = np.zeros(NC_PAD, np.float32)
        for c, tb in enumerate(cores):
            seeds = np.zeros((P, MEXP), dt)
            for g2 in range(NCH):
                sidx = tb["seedidx"][g2]
                pp, cc = np.where(sidx >= 0)
                seeds[pp, EB[g2] + sidx[pp, cc]] = seed_data[pp, cc]
            # segmented forward-fill scan: state = mask*state + seed (fp32
            # state, downcast per element) per chunk
            exp_t = np.zeros((P, MEXP), dt)
            rm = tb["runmask"].astype(np.float32)
            sd32 = seeds.astype(np.float32)
            for g2 in range(NCH):
                st = np.zeros(P, np.float32)
                for t in range(int(EB[g2]), int(EB[g2 + 1])):
                    st = rm[:, t] * st + sd32[:, t]
                    exp_t[:, t] = st.astype(dt)
            prod = (exp_t.astype(np.float32) * tb["w_exp"].astype(np.float32)).astype(dt)
            staging = np.zeros((P, 128 * T), dt)
            for ci, (g2, t0, t1) in enumerate(meta["r1_struct"]):
                idx = tb["idx1"][ci]
                data = prod[:, EB[g2]:EB[g2] + M1[g2]]
                pp, cc = np.where(idx >= 0)
                staging[pp, 128 * (TBASE[g2] + t0) + idx[pp, cc]] = data[pp, cc]
            t2 = np.zeros_like(staging)
            for t in range(T):
                t2[:, 128 * t:128 * (t + 1)] = staging[:, 128 * t:128 * (t + 1)].T
            slots = np.zeros((P, FD), dt)
            for g2 in range(NCH):
                idx = tb["idx2"][g2]
                data = t2[:, 128 * TBASE[g2]:128 * (TBASE[g2] + MTg[g2])]
                pp, cc = np.where(idx >= 0)
                slots[pp, g2 * CH + idx[pp, cc]] = data[pp, cc]
            NP2C, NP1C, SLOTC = meta["NP2C"], meta["NP1C"], meta["SLOTC"]
            sp = np.zeros((P, KP), np.float32)
            for g2 in range(NCH):
                ch = slots[:, g2 * CH:(g2 + 1) * CH].astype(np.float32)
                w2 = ch[:, :NP2C * 64].reshape(P, NP2C, 64).sum(axis=2)
                w1 = ch[:, NP2C * 64:].reshape(P, NP1C, 32).sum(axis=2)
                c0 = g2 * SLOTC
                sp[:, c0:c0 + NP2C] = w2
                sp[:, c0 + NP2C:c0 + SLOTC] = w1
            s = sp.astype(dt).astype(np.float32)[:, :Kreal]
            v = s + tb["b_in_t"]
            y32 = _act_np(v)
            jj, kk2 = np.meshgrid(np.arange(P), np.arange(Kreal), indexing="ij")
            y_next[SHARD * c + KMAX * jj.ravel() + kk2.ravel()] = y32.ravel()
        y = y_next
    return y


# ============================ BASS KERNEL ============================

def _build(cores, meta, n_iters, no_cc=False, skip_last_exchange=False):
    import concourse.bacc as bacc
    import concourse.mybir as mybir
    import concourse.tile as tile
    from concourse.masks import make_identity

    f16, f32, i16 = mybir.dt.float16, mybir.dt.float32, mybir.dt.int16
    AOP = mybir.AluOpType
    Kreal, KP, FD, NCH, CH = (meta["Kreal"], meta["KP"], meta["FD"],
                              meta["NCH"], meta["CH"])
    M1, EB, MTg, TBASE, T, MEXP, NR1 = (meta["M1"], meta["EB"], meta["MTg"],
                                        meta["TBASE"], meta["T"],
                                        meta["MEXP"], meta["NR1"])
    NP2C, NP1C, SLOTC = meta["NP2C"], meta["NP1C"], meta["SLOTC"]
    DSTW = [min(FD, (g + 1) * CH) - g * CH for g in range(NCH)]

    nc = bacc.Bacc("TRN2", target_bir_lowering=False)

    d_seed = [nc.dram_tensor(f"t_seed{g}", [P, SD], i16, kind="ExternalInput")
              for g in range(NCH)]
    d_rmask = nc.dram_tensor("t_rmask", [P, MEXP], f16, kind="ExternalInput")
    d_wexp = nc.dram_tensor("t_wexp", [P, MEXP], f16, kind="ExternalInput")
    d_idx1 = [nc.dram_tensor(f"t_idx1_{ci}", [P, int(M1[g2])], i16,
                             kind="ExternalInput")
              for ci, (g2, _, _) in enumerate(meta["r1_struct"])]
    d_idx2 = [nc.dram_tensor(f"t_idx2_{g}", [P, 128 * int(MTg[g])], i16,
                             kind="ExternalInput") for g in range(NCH)]
    d_bin = nc.dram_tensor("t_bin", [P, Kreal], f32, kind="ExternalInput")
    d_yout = nc.dram_tensor("y_out", [P, Kreal], f16, kind="ExternalOutput")
    d_ysh = nc.dram_tensor("y_shard", [1, SHARD], f16, kind="Internal")
    d_yfull = nc.dram_tensor("y_full", [1, NC_PAD], f16, kind="Internal",
                             addr_space="Shared")
    d_yin = nc.dram_tensor("y_in", [1, NC_PAD], f16, kind="ExternalInput")
    d_yall = nc.dram_tensor("y_all", [1, NC_PAD], f16, kind="ExternalOutput")

    with tile.TileContext(nc) as tc:
        with tc.tile_pool(name="tables", bufs=1) as tp, \
             tc.tile_pool(name="psum", bufs=8, space="PSUM") as pp:
            t_seed = [tp.tile([P, SD], i16, name=f"seed{g}") for g in range(NCH)]
            t_rmask = tp.tile([P, MEXP], f16, name="rmask")
            t_wexp = tp.tile([P, MEXP], f16, name="wexp")
            t_idx1 = [tp.tile([P, int(M1[g2])], i16, name=f"i1_{ci}")
                      for ci, (g2, _, _) in enumerate(meta["r1_struct"])]
            t_idx2 = [tp.tile([P, 128 * int(MTg[g])], i16, name=f"i2_{g}")
                      for g in range(NCH)]
            t_bin = tp.tile([P, Kreal], f32, name="bin")
            ident = tp.tile([P, P], f16, name="ident")
            y2d = tp.tile([P, QW], f16, name="y2d")
            expb = [tp.tile([P, int(M1[g])], f16, name=f"expb{g}")
                    for g in range(NCH)]
            seedb = [tp.tile([P, int(M1[g])], f16, name=f"seedb{g}")
                     for g in range(NCH)]
            stag = [tp.tile([P, 128 * int(MTg[g])], f16, name=f"stag{g}")
                    for g in range(NCH)]
            t2d = [tp.tile([P, 128 * int(MTg[g])], f16, name=f"t2d{g}")
                   for g in range(NCH)]
            slots = [tp.tile([P, DSTW[g]], f16, name=f"slots{g}")
                     for g in range(NCH)]
            sp = tp.tile([P, KP], f16, name="sp")
            vv = tp.tile([P, Kreal], f32, name="vv")
            y1b = tp.tile([P, Kreal], f32, name="y1b")
            rb = tp.tile([P, Kreal], f32, name="rb")
            mb = tp.tile([P, Kreal], mybir.dt.uint8, name="mb")
            y16 = tp.tile([P, KMAX], f16, name="y16")

            for g in range(NCH):
                nc.sync.dma_start(t_seed[g][:], d_seed[g][:])
                nc.sync.dma_start(t_idx2[g][:], d_idx2[g][:])
            nc.sync.dma_start(t_rmask[:], d_rmask[:])
            for ci in range(NR1):
                nc.sync.dma_start(t_idx1[ci][:], d_idx1[ci][:])
            nc.sync.dma_start(t_wexp[:], d_wexp[:])
            nc.sync.dma_start(t_bin[:], d_bin[:])
            make_identity(nc, ident[:])
            nc.sync.dma_start(y2d[:], d_yin[:].rearrange("o (p q) -> (o p) q", p=P))
            nc.vector.memset(y16[:], 0.0)

            r1_by_g = {}
            for ci, (g2, t0, t1) in enumerate(meta["r1_struct"]):
                r1_by_g.setdefault(g2, []).append((ci, t0, t1))

            # small chunk last: its short r1->copy->r2 chain ends the iteration
            g_order = [1, 0, 2] if NCH == 3 else list(range(NCH))

            def body(last=False):
                nbatch = 0
                for g in g_order:
                    w0, w1 = int(EB[g]), int(EB[g + 1])
                    mw = int(M1[g])
                    # seed run-starts for chunk g, then segmented forward-fill
                    nc.gpsimd.local_scatter(
                        seedb[g][:], y2d[:], t_seed[g][:],
                        channels=P, num_elems=mw, num_idxs=SD)
                    nc.vector.tensor_tensor_scan(
                        expb[g][:], t_rmask[:, w0:w1], seedb[g][:], 0.0,
                        op0=AOP.mult, op1=AOP.add)
                    nc.vector.tensor_tensor(expb[g][:], expb[g][:],
                                            t_wexp[:, w0:w1], op=AOP.mult)
                    # round 1 into per-chunk staging
                    for ci, t0, t1 in r1_by_g[g]:
                        nt = t1 - t0
                        nc.gpsimd.local_scatter(
                            stag[g][:, 128 * t0:128 * t1], expb[g][:],
                            t_idx1[ci][:], channels=P, num_elems=128 * nt,
                            num_idxs=mw)
                    # transposes; PSUM->SBUF copies alternate Act/DVE
                    Tg = int(MTg[g])
                    for tb0 in range(0, Tg, 8):
                        nb = min(8, Tg - tb0)
                        pt = pp.tile([P, 8 * P], f16, space="PSUM", tag="tr",
                                     name="tr")
                        for t in range(tb0, tb0 + nb):
                            nc.tensor.transpose(
                                pt[:, 128 * (t - tb0):128 * (t - tb0 + 1)],
                                stag[g][:, 128 * t:128 * (t + 1)], ident[:])
                        dst = t2d[g][:, 128 * tb0:128 * (tb0 + nb)]
                        if nbatch % 2 == 0:
                            nc.vector.tensor_copy(dst, pt[:, 0:128 * nb])
                        else:
                            nc.scalar.copy(dst, pt[:, 0:128 * nb])
                        nbatch += 1
                    # round 2 into dest slots
                    nc.gpsimd.local_scatter(
                        slots[g][:], t2d[g][:],
                        t_idx2[g][:], channels=P, num_elems=DSTW[g],
                        num_idxs=128 * Tg)
                    # segmented reduce: 64-wide slots for wide dests, then
                    # 32-wide (fp16 sums of |w*y|<0.2 stay O(1); validated
                    # against an fp64 reference)
                    c0 = g * SLOTC
                    n2w = NP2C * 64
                    with nc.allow_low_precision(reason="fp16 slot sums"):
                        nc.vector.tensor_reduce(
                            sp[:, c0:c0 + NP2C],
                            slots[g][:, 0:n2w].rearrange(
                                "p (k s) -> p k s", s=64),
                            axis=mybir.AxisListType.X, op=AOP.add)
                        nc.vector.tensor_reduce(
                            sp[:, c0 + NP2C:c0 + SLOTC],
                            slots[g][:, n2w:CH].rearrange(
                                "p (k s) -> p k s", s=32),
                            axis=mybir.AxisListType.X, op=AOP.add)
                nc.vector.tensor_tensor(vv[:], sp[:, 0:Kreal], t_bin[:], op=AOP.add)
                nc.vector.scalar_tensor_tensor(
                    y1b[:], vv[:], float(LEAK), vv[:], op0=AOP.mult, op1=AOP.max)
                nc.vector.tensor_scalar_max(rb[:], vv[:], 0.5)
                nc.vector.reciprocal(rb[:], rb[:])
                nc.vector.tensor_scalar(rb[:], rb[:], -0.25, 1.0,
                                        op0=AOP.mult, op1=AOP.add)
                nc.vector.tensor_scalar(mb[:], vv[:], 0.5, None, op0=AOP.is_gt)
                nc.vector.select(y16[:, 0:Kreal], mb[:], rb[:], y1b[:])
                if last:
                    return  # final shard never leaves this core pre-gather
                nc.sync.dma_start(
                    d_ysh[:].rearrange("o (p k) -> (o p) k", p=P), y16[:])
                if not no_cc:
                    nc.gpsimd.collective_compute(
                        "AllGather", AOP.bypass,
                        replica_groups=[list(range(NCORES))],
                        ins=[d_ysh[:]], outs=[d_yfull[:]])
                nc.sync.dma_start(
                    y2d[:], d_yfull[:].rearrange("o (p q) -> (o p) q", p=P))

            for it in range(n_iters):
                body(last=(skip_last_exchange and it == n_iters - 1))
            nc.sync.dma_start(d_yout[:], y16[:, 0:Kreal])
            nc.sync.dma_start(
                d_yall[:].rearrange("o (p q) -> (o p) q", p=P), y2d[:])

    nc.compile()
    return nc


def _in_maps(cores, meta):
    maps = []
    for tb in cores:
        m = {"t_wexp": tb["w_exp"], "t_bin": tb["b_in_t"],
             "t_rmask": tb["runmask"]}
        for g in range(meta["NCH"]):
            m[f"t_seed{g}"] = tb["seedidx"][g]
            m[f"t_idx2_{g}"] = tb["idx2"][g]
        for ci in range(meta["NR1"]):
            m[f"t_idx1_{ci}"] = tb["idx1"][ci]
        maps.append(m)
    return maps


def _gather_y(res, meta):
    Kreal = meta["Kreal"]
    y_full = np.zeros(NC_PAD, np.float32)
    jj, kk2 = np.meshgrid(np.arange(P), np.arange(Kreal), indexing="ij")
    for c in range(NCORES):
        y32 = res.results[c]["y_out"]
        y_full[SHARD * c + KMAX * jj.ravel() + kk2.ravel()] = y32.ravel()
    return y_full


SEG = 150  # whole run fits one NEFF


def kernel(**inputs):
    from concourse.bass_utils import run_bass_kernel_spmd
    inputs = {k: np.asarray(v) for k, v in inputs.items()}
    cores, perm, meta = _prep(**inputs)
    nseg = _ceil(ITERS, SEG)
    nc = _build(cores, meta, SEG, skip_last_exchange=(nseg == 1))
    maps = _in_maps(cores, meta)
    y_state = np.zeros((1, NC_PAD), np.float16)
    res = None
    for s in range(nseg):
        for m in maps:
            m["y_in"] = y_state
        res = run_bass_kernel_spmd(nc, [dict(m) for m in maps],
                                   core_ids=list(range(NCORES)))
        y_state = res.results[0]["y_all"]
    y_old = _gather_y(res, meta)[perm]
    out = (inputs["out_weights"].astype(np.float32)
           * y_old[inputs["out_indices"]])[None, :]
    return out.astype(np.float32)


if __name__ == "__main__":
    import sys, time
    sys.path.insert(0, "/root/problem")
    import reference
    inputs = {k: np.asarray(v) for k, v in reference.setup_inputs().items()}
    t0 = time.time()
    cores, perm, meta = _prep(**inputs)
    print(f"prep {time.time()-t0:.1f}s Kreal={meta['Kreal']} KP={meta['KP']} "
          f"FD={meta['FD']} M1={meta['M1']} MTg={meta['MTg']} T={meta['T']} "
          f"MEXP={meta['MEXP']} NR1={meta['NR1']}")
    if "sim" in sys.argv:
        n_it = int(sys.argv[sys.argv.index("sim") + 1]) if len(sys.argv) > 2 else 8
        import jax.numpy as jnp
        ni = np.asarray(jnp.zeros((N,), jnp.float32).at[jnp.asarray(inputs["in_indices"])].set(
            jnp.asarray(inputs["in_weights"], jnp.float32) * jnp.asarray(inputs["x"][0], jnp.float32)))
        b_in = (ni + inputs["biases"]).astype(np.float64)
        rw = inputs["rec_weights"].astype(np.float64)
        er, ec = inputs["edge_rows"], inputs["edge_cols"]
        yref = np.zeros(N, np.float64)
        for _ in range(n_it):
            s = np.bincount(er, weights=rw * yref[ec], minlength=N)
            v = s + b_in
            yref = np.where(v > 0.5, 1.0 - 0.25 / np.maximum(v, 0.5),
                            np.maximum(v, LEAK * v))
        scale = np.abs(yref).max()
        t0 = time.time()
        ys = _sim(cores, perm, meta, n_it, quant=False)
        print(f"sim(noquant,{n_it}) {time.time()-t0:.1f}s  max rel err:",
              np.abs(ys[perm] - yref).max() / scale)
        t0 = time.time()
        ysq = _sim(cores, perm, meta, n_it, quant=True)
        print(f"sim(fp16,{n_it}) {time.time()-t0:.1f}s  max rel err:",
              np.abs(ysq[perm] - yref).max() / scale)



# revision 37
# speedup vs baseline: 1.4656x; 1.0480x over previous
"""Bionetwork sparse-matvec recurrence on 8 trn2 NeuronCores.

y_{t+1} = act(A y_t + b_in), 150 iterations, A fixed sparse (3.2M edges,
100k nodes).  Dest-sharded across 8 cores; all routing tables SBUF-resident.

Per iteration, per core (local_scatter = vectorized GPSIMD within-row scatter):
  1. seed-scatter per dest-chunk g: canonical y -> run-starts of expansion
  2. segmented forward-fill via one tensor_tensor_scan (state=mask*state+seed)
  3. multiply by edge weights (fp16, in place)
  4. round-1 local_scatter: products -> staging tiles at col 128*t + dest_row
  5. PE transpose of each [128,128] staging tile (the cross-partition hop)
  6. round-2 local_scatter: transposed stream -> dest-slot layout
  7. segmented reduce straight into output order (per chunk: 15 64-wide
     slots for deg>32 dests + 18 32-wide slots; no fold pass)
  8. v = s + b_in; piecewise activation (select-fused); AllGather; reload y

Chunk assignment of dests is a greedy min-max balance (any dest fits any
chunk), flattening max edges per (chunk, src partition, dest row) -- that
max sets the staging-tile count T and the round-1/2 scatter cost.

Everything is table-driven; tables are built host-side from the (fixed)
edge lists and shipped as per-core input tensors to one shared program.
"""
import numpy as np

N = 100000
E = 3200000
P = 128
NCORES = 8
QW = 800                    # canonical width: 128*800 = 102400
NC_PAD = P * QW
SHARD = NC_PAD // NCORES    # 12800 = 128*100
KMAX = SHARD // P           # 100
ITERS = 150
LEAK = 0.01
RUN_CAP = 16                # fill rounds 1,2,4,8 cover runs of 16
SEED_REGIONS = 1
MAX_DST = 2046
TILES_PER_CALL = 15
SD = SEED_REGIONS * QW


def _ceil(a, b):
    return -(-a // b)


def _prep(x, in_weights, rec_weights, biases, out_weights,
          in_indices, edge_rows, edge_cols, out_indices):
    deg = np.bincount(edge_rows, minlength=N)
    assert deg.max() <= 64, f"max in-degree {deg.max()} > 64 unsupported"
    np2 = deg > 32  # wide dests get a 64-col slot, the rest a 32-col slot

    # Slot layout: every chunk holds NP2C 64-wide slots + NP1C 32-wide slots
    # per (core,row) bin, so ANY dest can be placed in ANY chunk.  A greedy
    # min-max pass then assigns dests to chunks to flatten the edge count per
    # (chunk, src partition, dest row) cell -- that max sets the staging tile
    # count T and with it the round-1/round-2 scatter cost.
    NP2C, NP1C = 15, 18
    NCH = 3
    CH = NP2C * 64 + NP1C * 32          # 1536
    FD = NCH * CH
    SLOTC = NP2C + NP1C                 # sp slots per chunk
    Kreal = KP = NCH * SLOTC            # 99
    assert Kreal <= KMAX
    NB = NCORES * P

    # deal dests round-robin over bins, np2 class first, LPT within class
    # (largest degree first improves the greedy min-max balance)
    rng = np.random.default_rng(12345)
    order = np.lexsort((rng.permutation(N), -deg, ~np2))
    i = np.arange(N)
    binid_pos = i % NB
    c_node = np.empty(N, np.int64)
    j_node = np.empty(N, np.int64)
    bin_node = np.empty(N, np.int64)
    c_node[order] = binid_pos % NCORES
    j_node[order] = binid_pos // NCORES
    bin_node[order] = binid_pos
    N2 = int(np2.sum())
    assert _ceil(N2, NB) <= NP2C * NCH and _ceil(N - N2, NB) <= NP1C * NCH
    rank_pos = np.empty(N, np.int64)
    rank_pos[order[:N2]] = np.arange(N2) // NB
    i1 = np.arange(N2, N)
    rank_pos[order[N2:]] = (i1 - N2 - ((i1 % NB) - N2) % NB) // NB
    r2max = int(rank_pos[order[:N2]].max()) + 1 if N2 else 0
    r1max = int(rank_pos[order[N2:]].max()) + 1

    # greedy chunk assignment, one round per (class, rank): each bin places
    # its rank-r dest into the chunk minimizing that bin-row's max cell
    p0_of_node = 16 * c_node + j_node // 8
    e_b = bin_node[edge_rows]
    e_p0 = p0_of_node[edge_cols]
    e_key = np.where(np2[edge_rows], 0, 256) + rank_pos[edge_rows]
    eo = np.argsort(e_key, kind="stable")
    e_key_s = e_key[eo]
    n_key = np.where(np2, 0, 256) + rank_pos
    no = np.argsort(n_key, kind="stable")
    n_key_s = n_key[no]

    cells = np.zeros((NCH, NB, P), np.int32)
    cnt2 = np.zeros((NCH, NB), np.int32)
    cnt1 = np.zeros((NCH, NB), np.int32)
    band_of = np.zeros(N, np.int64)
    kloc_of = np.zeros(N, np.int64)

    def _rounds(base_key, nmax, cnt, cap):
        for r in range(nmax):
            key = base_key + r
            na, nb_ = np.searchsorted(n_key_s, [key, key + 1])
            if na == nb_:
                continue
            nodes_r = no[na:nb_]
            ea, ebnd = np.searchsorted(e_key_s, [key, key + 1])
            H = np.zeros((NB, P), np.int32)
            if ea < ebnd:
                es = eo[ea:ebnd]
                np.add.at(H, (e_b[es], e_p0[es]), 1)
            cmax = (cells + H[None]).max(axis=2) * 64 + cnt
            cmax[cnt >= cap] = 1 << 30
            band = np.argmin(cmax, axis=0)
            bsel = bin_node[nodes_r]
            bb = band[bsel]
            band_of[nodes_r] = bb
            kloc_of[nodes_r] = cnt[bb, bsel]
            for ch in range(NCH):
                m = bsel[bb == ch]
                cells[ch, m] += H[m]
                cnt[ch, m] += 1

    _rounds(0, r2max, cnt2, NP2C)
    _rounds(256, r1max, cnt1, NP1C)

    k_node = band_of * SLOTC + np.where(np2, kloc_of, NP2C + kloc_of)
    perm = SHARD * c_node + KMAX * j_node + k_node

    # per-edge slot column: rank within dest (stable edge order)
    so = np.argsort(edge_rows, kind="stable")
    ds = edge_rows[so]
    st = np.r_[0, np.flatnonzero(np.diff(ds)) + 1]
    sid = np.zeros(E, np.int64)
    sid[st[1:]] = 1
    sid = np.cumsum(sid)
    e_drank = np.empty(E, np.int64)
    e_drank[so] = np.arange(E) - st[sid]
    e_np2 = np2[edge_rows]
    e_kloc = kloc_of[edge_rows]
    f_local = np.where(e_np2, 64 * e_kloc + e_drank,
                       NP2C * 64 + 32 * e_kloc + e_drank)
    fglob = band_of[edge_rows] * CH + f_local

    import jax.numpy as jnp
    node_in = np.asarray(
        jnp.zeros((N,), jnp.float32).at[jnp.asarray(in_indices)].set(
            jnp.asarray(in_weights, jnp.float32) * jnp.asarray(x[0], jnp.float32)))
    b_in_full = node_in + biases.astype(np.float32)

    dnew, snew = perm[edge_rows], perm[edge_cols]
    w_all = rec_weights.astype(np.float32)
    dcore = dnew // SHARD

    # ---------- pass 1: per-core edge geometry ----------
    geo = []
    for c in range(NCORES):
        em = np.where(dcore == c)[0]
        d_loc = dnew[em] - SHARD * c
        j, k = d_loc // KMAX, d_loc % KMAX
        s_new = snew[em]
        p0, q0 = s_new // QW, s_new % QW
        w = w_all[em]
        ne = em.size

        def ranks_of(key):
            so = np.argsort(key, kind="stable")
            ks = key[so]
            st = np.r_[0, np.flatnonzero(np.diff(ks)) + 1]
            sid = np.zeros(ne, np.int64)
            sid[st[1:]] = 1
            sid = np.cumsum(sid)
            r = np.arange(ne) - st[sid]
            out = np.empty(ne, np.int64)
            out[so] = r
            return out

        f = fglob[em]
        g = f // CH
        trank = ranks_of((g * P + p0) * P + j)
        # expansion position within (g,p0) ordered by q0, and rank within source
        so3 = np.lexsort((q0, p0, g))
        gp = (g * P + p0)[so3]
        st = np.r_[0, np.flatnonzero(np.diff(gp)) + 1]
        sid = np.zeros(ne, np.int64)
        sid[st[1:]] = 1
        sid = np.cumsum(sid)
        m_pos = np.empty(ne, np.int64)
        m_pos[so3] = np.arange(ne) - st[sid]
        gpq = ((g * P + p0) * QW + q0)[so3]
        st4 = np.r_[0, np.flatnonzero(np.diff(gpq)) + 1]
        sid4 = np.zeros(ne, np.int64)
        sid4[st4[1:]] = 1
        sid4 = np.cumsum(sid4)
        src_rank = np.empty(ne, np.int64)
        src_rank[so3] = np.arange(ne) - st4[sid4]
        geo.append(dict(j=j, p0=p0, q0=q0, w=w, f=f, g=g,
                        trank=trank, m_pos=m_pos, src_rank=src_rank, ne=ne))

    # uniform per-chunk sizes across cores
    M1 = np.zeros(NCH, np.int64)
    MTg = np.zeros(NCH, np.int64)
    for gg in geo:
        for g2 in range(NCH):
            sel = gg["g"] == g2
            if sel.any():
                M1[g2] = max(M1[g2], int(gg["m_pos"][sel].max()) + 1)
                MTg[g2] = max(MTg[g2], int(gg["trank"][sel].max()) + 1)
    M1 = (_ceil(M1, 2) * 2).astype(np.int64)
    EB = np.r_[0, np.cumsum(M1)]         # expansion bases
    MEXP = int(EB[-1])
    TBASE = np.r_[0, np.cumsum(MTg)]     # tile bases
    T = int(TBASE[-1])
    # round-1 call structure: (g, t0, t1), evenly-split windows <= 15 tiles
    r1_struct = []
    for g2 in range(NCH):
        tg = int(MTg[g2])
        ncall = _ceil(tg, TILES_PER_CALL)
        base, rem = divmod(tg, ncall)
        t0 = 0
        for ci in range(ncall):
            nt = base + (1 if ci < rem else 0)
            r1_struct.append((g2, t0, t0 + nt))
            t0 += nt
    NR1 = len(r1_struct)

    # ---------- pass 2: tables ----------
    cores = []
    for c in range(NCORES):
        gg = geo[c]
        j, p0, q0, w = gg["j"], gg["p0"], gg["q0"], gg["w"]
        f, g, trank, m_pos, src_rank = (gg["f"], gg["g"], gg["trank"],
                                        gg["m_pos"], gg["src_rank"])
        m_glob = EB[g] + m_pos
        dist = src_rank

        seedidx = np.full((NCH, P, SD), -1, np.int16)
        sm = dist == 0
        seedidx[g[sm], p0[sm], q0[sm]] = m_pos[sm].astype(np.int16)

        # scan fill mask: 1.0 inside a source run (copy state), 0.0 at starts
        runmask = np.zeros((P, MEXP), np.float16)
        mm = dist > 0
        runmask[p0[mm], m_glob[mm]] = 1.0

        w_exp = np.zeros((P, MEXP), np.float16)
        w_exp[p0, m_glob] = w.astype(np.float16)

        idx1 = []
        for (g2, t0, t1) in r1_struct:
            sel = (g == g2) & (trank >= t0) & (trank < t1)
            idx = np.full((P, int(M1[g2])), -1, np.int16)
            idx[p0[sel], m_pos[sel]] = (128 * (trank[sel] - t0) + j[sel]).astype(np.int16)
            idx1.append(idx)

        idx2 = []
        for g2 in range(NCH):
            sel = g == g2
            idx = np.full((P, 128 * int(MTg[g2])), -1, np.int16)
            idx[j[sel], 128 * trank[sel] + p0[sel]] = (f[sel] - g2 * CH).astype(np.int16)
            idx2.append(idx)

        b_in_t = np.zeros((P, Kreal), np.float32)
        nid = np.where((perm >= SHARD * c) & (perm < SHARD * (c + 1)))[0]
        dl = perm[nid] - SHARD * c
        b_in_t[dl // KMAX, dl % KMAX] = b_in_full[nid]

        cores.append(dict(seedidx=seedidx, runmask=runmask, w_exp=w_exp,
                          idx1=idx1, idx2=idx2, b_in_t=b_in_t))

    meta = dict(Kreal=Kreal, KP=KP, FD=FD, NCH=NCH, CH=CH, M1=M1, EB=EB,
                MTg=MTg, TBASE=TBASE, T=T, MEXP=MEXP, NR1=NR1,
                r1_struct=r1_struct, NP2C=NP2C, NP1C=NP1C, SLOTC=SLOTC)
    return cores, perm, meta


def _act_np(v):
    y1 = np.maximum(v, np.float32(LEAK) * v)
    ysat = (1.0 - 0.25 / np.maximum(v, 0.5)).astype(v.dtype)
    return np.where(v > 0.5, ysat, y1)


def _sim(cores, perm, meta, n_iters, quant=True):
    dt = np.float16 if quant else np.float32
    Kreal, KP, FD, NCH, CH = (meta["Kreal"], meta["KP"], meta["FD"],
                              meta["NCH"], meta["CH"])
    M1, EB, MTg, TBASE, T, MEXP = (meta["M1"], meta["EB"], meta["MTg"],
                                   meta["TBASE"], meta["T"], meta["MEXP"])
    y = np.zeros(NC_PAD, np.float32)
    for it in range(n_iters):
        y2d = y.reshape(P, QW).astype(dt)
        seed_data = y2d
        y_next = np.zeros(NC_PAD, np.float32)
        for c, tb in enumerate(cores):
            seeds = np.zeros((P, MEXP), dt)
            for g2 in range(NCH):
                sidx = tb["seedidx"][g2]
                pp, cc = np.where(sidx >= 0)
                seeds[pp, EB[g2] + sidx[pp, cc]] = seed_data[pp, cc]
            # segmented forward-fill scan: state = mask*state + seed (fp32
            # state, downcast per element) per chunk
            exp_t = np.zeros((P, MEXP), dt)
            rm = tb["runmask"].astype(np.float32)
            sd32 = seeds.astype(np.float32)
            for g2 in range(NCH):
                st = np.zeros(P, np.float32)
                for t in range(int(EB[g2]), int(EB[g2 + 1])):
                    st = rm[:, t] * st + sd32[:, t]
                    exp_t[:, t] = st.astype(dt)
            prod = (exp_t.astype(np.float32) * tb["w_exp"].astype(np.float32)).astype(dt)
            staging = np.zeros((P, 128 * T), dt)
            for ci, (g2, t0, t1) in enumerate(meta["r1_struct"]):
                idx = tb["idx1"][ci]
                data = prod[:, EB[g2]:EB[g2] + M1[g2]]
                pp, cc = np.where(idx >= 0)
                staging[pp, 128 * (TBASE[g2] + t0) + idx[pp, cc]] = data[pp, cc]
            t2 = np.zeros_like(staging)
            for t in range(T):
                t2[:, 128 * t:128 * (t + 1)] = staging[:, 128 * t:128 * (t + 1)].T
            slots = np.zeros((P, FD), dt)
            for g2 in range(NCH):
                idx = tb["idx2"][g2]
                data = t2[:, 128 * TBASE[g2]:128 * (TBASE[g2] + MTg[g2])]
                pp, cc = np.where(idx >= 0)
                slots[pp, g2 * CH + idx[pp, cc]] = data[pp, cc]
            NP2C, NP1C, SLOTC = meta["NP2C"], meta["NP1C"], meta["SLOTC"]
            sp = np.zeros((P, KP), np.float32)
            for g2 in range(NCH):
                ch = slots[:, g2 * CH:(g2 + 1) * CH].astype(np.float32)
                w2 = ch[:, :NP2C * 64].reshape(P, NP2C, 64).sum(axis=2)
                w1 = ch[:, NP2C * 64:].reshape(P, NP1C, 32).sum(axis=2)
                c0 = g2 * SLOTC
                sp[:, c0:c0 + NP2C] = w2
                sp[:, c0 + NP2C:c0 + SLOTC] = w1
            s = sp.astype(dt).astype(np.float32)[:, :Kreal]
            v = s + tb["b_in_t"]
            y32 = _act_np(v)
            jj, kk2 = np.meshgrid(np.arange(P), np.arange(Kreal), indexing="ij")
            y_next[SHARD * c + KMAX * jj.ravel() + kk2.ravel()] = y32.ravel()
        y = y_next
    return y


# ============================ BASS KERNEL ============================

def _build(cores, meta, n_iters, no_cc=False, skip_last_exchange=False):
    import concourse.bacc as bacc
    import concourse.mybir as mybir
    import concourse.tile as tile
    from concourse.masks import make_identity

    f16, f32, i16 = mybir.dt.float16, mybir.dt.float32, mybir.dt.int16
    AOP = mybir.AluOpType
    Kreal, KP, FD, NCH, CH = (meta["Kreal"], meta["KP"], meta["FD"],
                              meta["NCH"], meta["CH"])
    M1, EB, MTg, TBASE, T, MEXP, NR1 = (meta["M1"], meta["EB"], meta["MTg"],
                                        meta["TBASE"], meta["T"],
                                        meta["MEXP"], meta["NR1"])
    NP2C, NP1C, SLOTC = meta["NP2C"], meta["NP1C"], meta["SLOTC"]
    DSTW = [min(FD, (g + 1) * CH) - g * CH for g in range(NCH)]

    nc = bacc.Bacc("TRN2", target_bir_lowering=False)

    d_seed = [nc.dram_tensor(f"t_seed{g}", [P, SD], i16, kind="ExternalInput")
              for g in range(NCH)]
    d_rmask = nc.dram_tensor("t_rmask", [P, MEXP], f16, kind="ExternalInput")
    d_wexp = nc.dram_tensor("t_wexp", [P, MEXP], f16, kind="ExternalInput")
    d_idx1 = [nc.dram_tensor(f"t_idx1_{ci}", [P, int(M1[g2])], i16,
                             kind="ExternalInput")
              for ci, (g2, _, _) in enumerate(meta["r1_struct"])]
    d_idx2 = [nc.dram_tensor(f"t_idx2_{g}", [P, 128 * int(MTg[g])], i16,
                             kind="ExternalInput") for g in range(NCH)]
    d_bin = nc.dram_tensor("t_bin", [P, Kreal], f32, kind="ExternalInput")
    d_yout = nc.dram_tensor("y_out", [P, Kreal], f16, kind="ExternalOutput")
    d_ysh = nc.dram_tensor("y_shard", [1, SHARD], f16, kind="Internal")
    d_yfull = nc.dram_tensor("y_full", [1, NC_PAD], f16, kind="Internal",
                             addr_space="Shared")
    d_yin = nc.dram_tensor("y_in", [1, NC_PAD], f16, kind="ExternalInput")
    d_yall = nc.dram_tensor("y_all", [1, NC_PAD], f16, kind="ExternalOutput")

    with tile.TileContext(nc) as tc:
        with tc.tile_pool(name="tables", bufs=1) as tp, \
             tc.tile_pool(name="psum", bufs=8, space="PSUM") as pp:
            t_seed = [tp.tile([P, SD], i16, name=f"seed{g}") for g in range(NCH)]
            t_rmask = tp.tile([P, MEXP], f16, name="rmask")
            t_wexp = tp.tile([P, MEXP], f16, name="wexp")
            t_idx1 = [tp.tile([P, int(M1[g2])], i16, name=f"i1_{ci}")
                      for ci, (g2, _, _) in enumerate(meta["r1_struct"])]
            t_idx2 = [tp.tile([P, 128 * int(MTg[g])], i16, name=f"i2_{g}")
                      for g in range(NCH)]
            t_bin = tp.tile([P, Kreal], f32, name="bin")
            ident = tp.tile([P, P], f16, name="ident")
            y2d = tp.tile([P, QW], f16, name="y2d")
            expb = [tp.tile([P, int(M1[g])], f16, name=f"expb{g}")
                    for g in range(NCH)]
            seedb = [tp.tile([P, int(M1[g])], f16, name=f"seedb{g}")
                     for g in range(NCH)]
            stag = [tp.tile([P, 128 * int(MTg[g])], f16, name=f"stag{g}")
                    for g in range(NCH)]
            t2d = [tp.tile([P, 128 * int(MTg[g])], f16, name=f"t2d{g}")
                   for g in range(NCH)]
            slots = [tp.tile([P, DSTW[g]], f16, name=f"slots{g}")
                     for g in range(NCH)]
            sp = tp.tile([P, KP], f16, name="sp")
            vv = tp.tile([P, Kreal], f32, name="vv")
            y1b = tp.tile([P, Kreal], f32, name="y1b")
            rb = tp.tile([P, Kreal], f32, name="rb")
            mb = tp.tile([P, Kreal], mybir.dt.uint8, name="mb")
            y16 = tp.tile([P, KMAX], f16, name="y16")

            for g in range(NCH):
                nc.sync.dma_start(t_seed[g][:], d_seed[g][:])
                nc.sync.dma_start(t_idx2[g][:], d_idx2[g][:])
            nc.sync.dma_start(t_rmask[:], d_rmask[:])
            for ci in range(NR1):
                nc.sync.dma_start(t_idx1[ci][:], d_idx1[ci][:])
            nc.sync.dma_start(t_wexp[:], d_wexp[:])
            nc.sync.dma_start(t_bin[:], d_bin[:])
            make_identity(nc, ident[:])
            nc.sync.dma_start(y2d[:], d_yin[:].rearrange("o (p q) -> (o p) q", p=P))
            nc.vector.memset(y16[:], 0.0)

            r1_by_g = {}
            for ci, (g2, t0, t1) in enumerate(meta["r1_struct"]):
                r1_by_g.setdefault(g2, []).append((ci, t0, t1))

            # small chunk last: its short r1->copy->r2 chain ends the iteration
            g_order = [1, 0, 2] if NCH == 3 else list(range(NCH))

            def body(last=False):
                # phase 1: seeds, scan-fill, weight mult, round-1 scatters --
                # all chunks' r1 calls queue on Pool ahead of any r2, so the
                # last chunk's staging is ready before Pool reaches its r2
                for g in g_order:
                    w0, w1 = int(EB[g]), int(EB[g + 1])
                    mw = int(M1[g])
                    nc.gpsimd.local_scatter(
                        seedb[g][:], y2d[:], t_seed[g][:],
                        channels=P, num_elems=mw, num_idxs=SD)
                    nc.vector.tensor_tensor_scan(
                        expb[g][:], t_rmask[:, w0:w1], seedb[g][:], 0.0,
                        op0=AOP.mult, op1=AOP.add)
                    nc.vector.tensor_tensor(expb[g][:], expb[g][:],
                                            t_wexp[:, w0:w1], op=AOP.mult)
                    for ci, t0, t1 in r1_by_g[g]:
                        nt = t1 - t0
                        nc.gpsimd.local_scatter(
                            stag[g][:, 128 * t0:128 * t1], expb[g][:],
                            t_idx1[ci][:], channels=P, num_elems=128 * nt,
                            num_idxs=mw)
                # phase 2: transposes; PSUM->SBUF copies alternate DVE/Act
                nbatch = 0
                for g in g_order:
                    Tg = int(MTg[g])
                    for tb0 in range(0, Tg, 8):
                        nb = min(8, Tg - tb0)
                        pt = pp.tile([P, 8 * P], f16, space="PSUM", tag="tr",
                                     name="tr")
                        for t in range(tb0, tb0 + nb):
                            nc.tensor.transpose(
                                pt[:, 128 * (t - tb0):128 * (t - tb0 + 1)],
                                stag[g][:, 128 * t:128 * (t + 1)], ident[:])
                        dst = t2d[g][:, 128 * tb0:128 * (tb0 + nb)]
                        if nbatch % 2 == 0:
                            nc.vector.tensor_copy(dst, pt[:, 0:128 * nb])
                        else:
                            nc.scalar.copy(dst, pt[:, 0:128 * nb])
                        nbatch += 1
                # phase 3: round-2 scatters + segmented reduces (64-wide
                # slots for wide dests then 32-wide; fp16 sums of |w*y|<0.2
                # stay O(1), validated against an fp64 reference)
                for g in g_order:
                    nc.gpsimd.local_scatter(
                        slots[g][:], t2d[g][:],
                        t_idx2[g][:], channels=P, num_elems=DSTW[g],
                        num_idxs=128 * int(MTg[g]))
                    c0 = g * SLOTC
                    n2w = NP2C * 64
                    with nc.allow_low_precision(reason="fp16 slot sums"):
                        nc.vector.tensor_reduce(
                            sp[:, c0:c0 + NP2C],
                            slots[g][:, 0:n2w].rearrange(
                                "p (k s) -> p k s", s=64),
                            axis=mybir.AxisListType.X, op=AOP.add)
                        nc.vector.tensor_reduce(
                            sp[:, c0 + NP2C:c0 + SLOTC],
                            slots[g][:, n2w:CH].rearrange(
                                "p (k s) -> p k s", s=32),
                            axis=mybir.AxisListType.X, op=AOP.add)
                nc.vector.tensor_tensor(vv[:], sp[:, 0:Kreal], t_bin[:], op=AOP.add)
                nc.vector.scalar_tensor_tensor(
                    y1b[:], vv[:], float(LEAK), vv[:], op0=AOP.mult, op1=AOP.max)
                nc.vector.tensor_scalar_max(rb[:], vv[:], 0.5)
                nc.vector.reciprocal(rb[:], rb[:])
                nc.vector.tensor_scalar(rb[:], rb[:], -0.25, 1.0,
                                        op0=AOP.mult, op1=AOP.add)
                nc.vector.tensor_scalar(mb[:], vv[:], 0.5, None, op0=AOP.is_gt)
                nc.vector.select(y16[:, 0:Kreal], mb[:], rb[:], y1b[:])
                if last:
                    return  # final shard never leaves this core pre-gather
                nc.sync.dma_start(
                    d_ysh[:].rearrange("o (p k) -> (o p) k", p=P), y16[:])
                if not no_cc:
                    nc.gpsimd.collective_compute(
                        "AllGather", AOP.bypass,
                        replica_groups=[list(range(NCORES))],
                        ins=[d_ysh[:]], outs=[d_yfull[:]])
                nc.sync.dma_start(
                    y2d[:], d_yfull[:].rearrange("o (p q) -> (o p) q", p=P))

            for it in range(n_iters):
                body(last=(skip_last_exchange and it == n_iters - 1))
            nc.sync.dma_start(d_yout[:], y16[:, 0:Kreal])
            nc.sync.dma_start(
                d_yall[:].rearrange("o (p q) -> (o p) q", p=P), y2d[:])

    nc.compile()
    return nc


def _in_maps(cores, meta):
    maps = []
    for tb in cores:
        m = {"t_wexp": tb["w_exp"], "t_bin": tb["b_in_t"],
             "t_rmask": tb["runmask"]}
        for g in range(meta["NCH"]):
            m[f"t_seed{g}"] = tb["seedidx"][g]
            m[f"t_idx2_{g}"] = tb["idx2"][g]
        for ci in range(meta["NR1"]):
            m[f"t_idx1_{ci}"] = tb["idx1"][ci]
        maps.append(m)
    return maps


def _gather_y(res, meta):
    Kreal = meta["Kreal"]
    y_full = np.zeros(NC_PAD, np.float32)
    jj, kk2 = np.meshgrid(np.arange(P), np.arange(Kreal), indexing="ij")
    for c in range(NCORES):
        y32 = res.results[c]["y_out"]
        y_full[SHARD * c + KMAX * jj.ravel() + kk2.ravel()] = y32.ravel()
    return y_full


SEG = 150  # whole run fits one NEFF


def kernel(**inputs):
    from concourse.bass_utils import run_bass_kernel_spmd
    inputs = {k: np.asarray(v) for k, v in inputs.items()}
    cores, perm, meta = _prep(**inputs)
    nseg = _ceil(ITERS, SEG)
    nc = _build(cores, meta, SEG, skip_last_exchange=(nseg == 1))
    maps = _in_maps(cores, meta)
    y_state = np.zeros((1, NC_PAD), np.float16)
    res = None
    for s in range(nseg):
        for m in maps:
            m["y_in"] = y_state
        res = run_bass_kernel_spmd(nc, [dict(m) for m in maps],
                                   core_ids=list(range(NCORES)))
        y_state = res.results[0]["y_all"]
    y_old = _gather_y(res, meta)[perm]
    out = (inputs["out_weights"].astype(np.float32)
           * y_old[inputs["out_indices"]])[None, :]
    return out.astype(np.float32)


if __name__ == "__main__":
    import sys, time
    sys.path.insert(0, "/root/problem")
    import reference
    inputs = {k: np.asarray(v) for k, v in reference.setup_inputs().items()}
    t0 = time.time()
    cores, perm, meta = _prep(**inputs)
    print(f"prep {time.time()-t0:.1f}s Kreal={meta['Kreal']} KP={meta['KP']} "
          f"FD={meta['FD']} M1={meta['M1']} MTg={meta['MTg']} T={meta['T']} "
          f"MEXP={meta['MEXP']} NR1={meta['NR1']}")
    if "sim" in sys.argv:
        n_it = int(sys.argv[sys.argv.index("sim") + 1]) if len(sys.argv) > 2 else 8
        import jax.numpy as jnp
        ni = np.asarray(jnp.zeros((N,), jnp.float32).at[jnp.asarray(inputs["in_indices"])].set(
            jnp.asarray(inputs["in_weights"], jnp.float32) * jnp.asarray(inputs["x"][0], jnp.float32)))
        b_in = (ni + inputs["biases"]).astype(np.float64)
        rw = inputs["rec_weights"].astype(np.float64)
        er, ec = inputs["edge_rows"], inputs["edge_cols"]
        yref = np.zeros(N, np.float64)
        for _ in range(n_it):
            s = np.bincount(er, weights=rw * yref[ec], minlength=N)
            v = s + b_in
            yref = np.where(v > 0.5, 1.0 - 0.25 / np.maximum(v, 0.5),
                            np.maximum(v, LEAK * v))
        scale = np.abs(yref).max()
        t0 = time.time()
        ys = _sim(cores, perm, meta, n_it, quant=False)
        print(f"sim(noquant,{n_it}) {time.time()-t0:.1f}s  max rel err:",
              np.abs(ys[perm] - yref).max() / scale)
        t0 = time.time()
        ysq = _sim(cores, perm, meta, n_it, quant=True)
        print(f"sim(fp16,{n_it}) {time.time()-t0:.1f}s  max rel err:",
              np.abs(ysq[perm] - yref).max() / scale)

